# revision 72
# baseline (speedup 1.0000x reference)
"""DeepSeekV3 block (MLA attention + top-2-of-8 MoE) on 8 trn2 NeuronCores.

Sharding: cores 0-3 -> batch 0, cores 4-7 -> batch 1. Within a batch group
of 4 cores, each core owns S/4 query tokens chosen as SL strided 128-row
blocks ordered by causal depth (blocks r+12, r+8, r+4, r for sub-rank r at
S=2048), which makes the flash-attention k-loop narrow uniformly across
cores: one SPMD program, all per-core differences live in input data.
k/v/latent projections are recomputed per core (replicated within the
batch group) to avoid collectives.

v2 changes vs baseline:
- q projection runs first (overlaps the xT window DMAs); window loop is
  double-buffered and fuses the per-window v up-projection.
- flash attention is software-pipelined: scores for block j+1 are issued
  ahead of the AV matmuls for block j so the in-order tensor queue never
  stalls on the softmax exp; the epilogue reciprocal runs directly on the
  PSUM row (no DMA round trip) and O2 is double-banked so the next head
  pair starts immediately.
- router matmul is reoriented (small wr stationary, tokens streamed) with
  the gating math vectorized over all four 128-token blocks at once.
- MoE expert matmuls run in fp8 (e4m3) with DoubleRow packing: weights are
  pre-scaled/interleaved on the host, activations are quantized on-chip,
  and all scale factors fold into existing activation/broadcast ops.
  Gate values are folded into the hidden states via a gpsimd partition
  broadcast instead of a tensor-engine broadcast matmul.

Layout convention: activations are kept transposed [feature, token] so
weight matrices are always the stationary matmul operand, and softmax
denominators come from a ones column appended to the value tiles.
"""

import sys

sys.path.insert(0, "/opt/trn_rl_repo")

from contextlib import ExitStack

import ml_dtypes
import numpy as np

import concourse.bass as bass
import concourse.tile as tile
from concourse import bacc
from concourse import mybir
from concourse.bass_utils import run_bass_kernel_spmd

F32 = mybir.dt.float32
BF16 = mybir.dt.bfloat16
FP8 = mybir.dt.float8e4
AF = mybir.ActivationFunctionType
ALU = mybir.AluOpType
BF = ml_dtypes.bfloat16
F8 = ml_dtypes.float8_e4m3

B, D = 2, 1024
H, HD = 16, 64
HALF = HD // 2
R = 256
E, TOPK, MH = 8, 2, 256
EPS = 1e-6
THETA = 10000.0
P = 128
NCORES = 8

# fp8 scale plan for the MoE: every scale is a power of two so the
# compensations fold exactly into activation scales.
S_X = 32.0        # x2n -> fp8
S_W13 = 2048.0    # w1/w3 -> fp8
S_W2 = 2048.0     # w2 -> fp8
S_H = 8.0         # hidden*gate -> fp8
C_SILU = 1.0 / (S_X * S_W13)          # descale inside the silu activation
C_GB = S_H / (S_X * S_W13)            # folded into the gate broadcast
C_OUT = 1.0 / (S_H * S_W2)            # final descale before the residual


def _build(S: int):
    NB = S // P               # seq blocks per batch (16 at S=2048)
    SL = NB // 4              # q-block slots per core
    TOK = SL * P              # own tokens per core
    WIN = min(512, S)
    NW = S // WIN
    NHP = H // 2              # 8 head pairs
    DCH = D // P              # 8
    RCH = R // P              # 2
    HD1 = HD + 1

    nc = bacc.Bacc(None, target_bir_lowering=False)

    xT = nc.dram_tensor("xT", [D, S], F32, kind="ExternalInput")
    xTq = nc.dram_tensor("xTq", [D, TOK], F32, kind="ExternalInput")
    cos4k = nc.dram_tensor("cos4k", [P, S], BF16, kind="ExternalInput")
    sin4kn = nc.dram_tensor("sin4kn", [P, S], BF16, kind="ExternalInput")
    cos4q = nc.dram_tensor("cos4q", [P, TOK], BF16, kind="ExternalInput")
    sin4qn = nc.dram_tensor("sin4qn", [P, TOK], BF16, kind="ExternalInput")
    maskt = nc.dram_tensor("maskt", [NB, 2, P, P], BF16, kind="ExternalInput")
    wqn = nc.dram_tensor("wqn", [D, H * HD], BF16, kind="ExternalInput")
    wdkvn = nc.dram_tensor("wdkvn", [D, R], BF16, kind="ExternalInput")
    wukx = nc.dram_tensor("wukx", [R, 2, H * HD], BF16, kind="ExternalInput")
    wuv = nc.dram_tensor("wuv", [R, H * HD], BF16, kind="ExternalInput")
    wo = nc.dram_tensor("wo", [H * HD, D], BF16, kind="ExternalInput")
    wrn = nc.dram_tensor("wrn", [D, E], F32, kind="ExternalInput")
    bias128 = nc.dram_tensor("bias128", [P, E], F32, kind="ExternalInput")
    w13dr = nc.dram_tensor("w13dr", [E, DCH // 2, P, 2, 2 * MH], FP8,
                           kind="ExternalInput")
    w2dr = nc.dram_tensor("w2dr", [E, P, 2, D], FP8, kind="ExternalInput")
    identf = nc.dram_tensor("identf", [P, P], F32, kind="ExternalInput")
    outT = nc.dram_tensor("outT", [D, TOK], F32, kind="ExternalOutput")

    with tile.TileContext(nc) as tc, ExitStack() as ctx:
        p_const = ctx.enter_context(tc.tile_pool(name="const", bufs=1))
        p_x2 = ctx.enter_context(tc.tile_pool(name="x2", bufs=1))

        ones_bf = p_const.tile([P, 1], BF16, tag="ones_bf", name="ones_bf")
        nc.vector.memset(ones_bf, 1.0)
        eps1 = p_const.tile([1, 1], F32, tag="eps1", name="eps1")
        nc.vector.memset(eps1, EPS)
        ones_row = p_const.tile([1, HD], F32, tag="ones_row", name="ones_row")
        nc.vector.memset(ones_row, 1.0)
        sb_ident = p_const.tile([P, P], F32, tag="ident", name="ident")
        nc.sync.dma_start(sb_ident, identf[:, :])
        ident_bf = p_const.tile([P, P], BF16, tag="identb", name="identb")
        nc.scalar.copy(ident_bf, sb_ident)

        # own-token x loads first: the q projection consumes them and runs
        # while the full-sequence windows stream in behind.
        sb_xq = []
        for dch in range(DCH):
            t = p_x2.tile([P, TOK], F32, tag=f"xq{dch}", name=f"xq{dch}")
            nc.sync.dma_start(t, xTq[dch * P:(dch + 1) * P, :])
            sb_xq.append(t)

        # all big loads share the sync queue in explicit first-use order;
        # tiles are declared here, the DMAs are interleaved below
        p_wk = ctx.enter_context(tc.tile_pool(name="wk", bufs=1))
        sb_wdkv = p_wk.tile([P, DCH, R], BF16, tag="wdkv", name="wdkv")
        sb_wuv = p_wk.tile([P, RCH, H * HD], BF16, tag="wuv", name="wuv")
        sb_wuk = p_wk.tile([P, RCH, 2, H * HD], BF16, tag="wuk", name="wuk")
        sb_cos4k = p_wk.tile([P, S], BF16, tag="cos4k", name="cos4k")
        sb_sin4kn = p_wk.tile([P, S], BF16, tag="sin4kn", name="sin4kn")

        def rmsnorm_cols(pool, ppool, src_tiles, ncols, nametag):
            """src_tiles: DCH sbuf [P, ncols] f32 -> DCH bf16 tiles,
            rms-normalized across the full d axis. Also returns the psum
            broadcast of 1/rms for fp32 consumers."""
            sq = []
            for dch in range(DCH):
                t = pool.tile([P, ncols], BF16, tag=f"sq{dch % 2}",
                              name=f"sq{dch % 2}")
                nc.scalar.activation(t, src_tiles[dch], AF.Square)
                sq.append(t)
            ss = ppool.tile([P, ncols], F32, tag="ss", name="ss")
            for dch in range(DCH):
                nc.tensor.matmul(ss[0:1, :], ones_bf, sq[dch],
                                 start=(dch == 0), stop=(dch == DCH - 1))
            sd = pool.tile([1, ncols], F32, tag="sd", name="sd")
            nc.scalar.activation(sd, ss[0:1, :], AF.Sqrt,
                                 bias=eps1, scale=1.0 / D)
            sdw = pool.tile([P, ncols // P], F32, tag="sdw", name="sdw")
            nc.sync.dma_start(sdw, sd)
            rcw = pool.tile([P, ncols // P], F32, tag="rcw", name="rcw")
            nc.vector.reciprocal(rcw, sdw)
            rsv = pool.tile([1, ncols], F32, tag="rsv", name="rsv")
            nc.sync.dma_start(rsv, rcw)
            rsb = pool.tile([P, ncols], F32, tag="rsb_s", name="rsb_s")
            nc.gpsimd.partition_broadcast(rsb, rsv)
            out = []
            for dch in range(DCH):
                t = pool.tile([P, ncols], BF16, tag=f"h_{nametag}{dch}",
                              name=f"h_{nametag}{dch}")
                nc.vector.tensor_tensor(t, src_tiles[dch], rsb, ALU.mult)
                out.append(t)
            return out, rsb

        def rope6(pool, pre_ps, cos_t, sin_t, out_tile, nametag):
            """rope on psum [P, cols] (2 heads stacked) -> bf16 out_tile.
            Engines are partition-lane-locked, so the half-swap goes
            through SBUF->SBUF DMA."""
            kbf = pool.tile(list(out_tile.shape), BF16, tag=f"rkb_{nametag}",
                            name=f"rkb_{nametag}")
            nc.vector.tensor_copy(kbf, pre_ps)
            ksw = pool.tile(list(out_tile.shape), BF16, tag=f"rsw_{nametag}",
                            name=f"rsw_{nametag}")
            # gpsimd queue: keeps the partition swap off the sync queue,
            # which carries the latency-critical input loads
            for g in range(4):
                a = g * HALF
                pa = (g + 1) * HALF if g % 2 == 0 else (g - 1) * HALF
                nc.gpsimd.dma_start(ksw[a:a + HALF, :], kbf[pa:pa + HALF, :])
            tmp = pool.tile(list(out_tile.shape), BF16, tag=f"rtm_{nametag}",
                            name=f"rtm_{nametag}")
            nc.vector.tensor_tensor(tmp, ksw, sin_t, ALU.mult)
            nc.vector.tensor_tensor(out_tile, kbf, cos_t, ALU.mult)
            nc.vector.tensor_tensor(out_tile, out_tile, tmp, ALU.add)

        attnT = [p_x2.tile([P, TOK], BF16, tag=f"attnT{i}", name=f"attnT{i}")
                 for i in range(NHP)]
        qTa = [p_x2.tile([P, TOK], BF16, tag=f"qTa{i}", name=f"qTa{i}")
               for i in range(NHP)]

        with ExitStack() as kvctx:
            p_kv = kvctx.enter_context(tc.tile_pool(name="kv", bufs=1))
            vext = [p_kv.tile([P, H * HD1], BF16, tag=f"vext{i}",
                              name=f"vext{i}") for i in range(NB)]
            cT = [p_kv.tile([P, S], BF16, tag=f"cT{i}", name=f"cT{i}")
                  for i in range(RCH)]

            # ---- phase A: qT + rope (own tokens; overlaps window DMAs) ----
            with ExitStack() as s5:
                p_q = s5.enter_context(tc.tile_pool(name="q", bufs=2))
                p_wq = s5.enter_context(tc.tile_pool(name="wqp", bufs=1))
                pp_5 = s5.enter_context(
                    tc.tile_pool(name="p5", bufs=2, space="PSUM"))
                sb_wq = p_wq.tile([P, DCH, H * HD], BF16, tag="wq", name="wq")
                nc.sync.dma_start(
                    sb_wq, wqn[:, :].rearrange("(c p) n -> p c n", p=P))
                sb_cos4q = p_wq.tile([P, TOK], BF16, tag="cos4q",
                                     name="cos4q")
                nc.sync.dma_start(sb_cos4q, cos4q[:, :])
                sb_sin4qn = p_wq.tile([P, TOK], BF16, tag="sin4qn",
                                      name="sin4qn")
                nc.sync.dma_start(sb_sin4qn, sin4qn[:, :])
                h1q, _ = rmsnorm_cols(p_q, pp_5, sb_xq, TOK, "nq")
                for hp in range(NHP):
                    hc = hp * 2 * HD
                    qps = pp_5.tile([P, TOK], F32, tag="mm", name="mm")
                    for dch in range(DCH):
                        nc.tensor.matmul(
                            qps, sb_wq[:, dch, hc:hc + P], h1q[dch],
                            start=(dch == 0), stop=(dch == DCH - 1))
                    rope6(p_q, qps, sb_cos4q, sb_sin4qn, qTa[hp], "q")

            p_kt = kvctx.enter_context(tc.tile_pool(name="kt", bufs=2))

            def em_kt_win(kt, khp, w, ppool, ptag, pbufs):
                hc = khp * 2 * HD
                c0 = w * WIN
                if pbufs is None:
                    kps = ppool.tile([P, 2, 512], F32, tag=ptag, name=ptag)
                else:
                    kps = ppool.tile([P, 2, 512], F32, tag=ptag, name=ptag,
                                     bufs=pbufs)
                # sw=0: k, sw=1: half-swapped k (weights permuted
                # host-side) -> rope without any partition moves
                for sw in range(2):
                    for rch in range(RCH):
                        nc.tensor.matmul(
                            kps[:, sw, 0:WIN],
                            sb_wuk[:, rch, sw, hc:hc + P],
                            cT[rch][:, c0:c0 + WIN],
                            start=(rch == 0), stop=(rch == RCH - 1))
                kbf2 = p_kt.tile([P, 2, WIN], BF16, tag="kbf2", name="kbf2")
                nc.vector.tensor_copy(kbf2, kps[:, :, 0:WIN])
                ktmp = p_kt.tile([P, WIN], BF16, tag="ktmp", name="ktmp")
                nc.vector.tensor_tensor(ktmp, kbf2[:, 1, :],
                                        sb_sin4kn[:, c0:c0 + WIN], ALU.mult)
                nc.vector.tensor_tensor(kt[:, c0:c0 + WIN], kbf2[:, 0, :],
                                        sb_cos4k[:, c0:c0 + WIN], ALU.mult)
                nc.vector.tensor_tensor(kt[:, c0:c0 + WIN],
                                        kt[:, c0:c0 + WIN], ktmp, ALU.add)

            kt0 = p_kt.tile([P, S], BF16, tag="kTa", name="kTa")

            # ---- phase B: per window rmsnorm -> latent cT -> v up;
            # head pair 0's kT is built as each window's cT lands ----
            with ExitStack() as s12:
                p_xw = s12.enter_context(tc.tile_pool(name="xw", bufs=2))
                p_n1 = s12.enter_context(tc.tile_pool(name="n1", bufs=2))
                pp_12 = s12.enter_context(
                    tc.tile_pool(name="p12", bufs=2, space="PSUM"))

                def em_xw(w):
                    c0 = w * WIN
                    xw = []
                    for dch in range(DCH):
                        t = p_xw.tile([P, WIN], F32, tag=f"xw{dch}",
                                      name=f"xw{dch}")
                        nc.sync.dma_start(
                            t, xT[dch * P:(dch + 1) * P, c0:c0 + WIN])
                        xw.append(t)
                    return xw

                # windows 0/1 load right behind the phase-A inputs; the
                # flash-phase weights are interleaved between the remaining
                # windows so everything lands just before first use
                xw_pre = [em_xw(0), em_xw(1)]
                nc.sync.dma_start(
                    sb_wdkv, wdkvn[:, :].rearrange("(c p) r -> p c r", p=P))
                nc.sync.dma_start(
                    sb_wuv, wuv[:, :].rearrange("(c p) n -> p c n", p=P))
                nc.sync.dma_start(
                    sb_wuk,
                    wukx[:, :, :].rearrange("(c p) s n -> p c s n", p=P))
                for w in range(NW):
                    c0 = w * WIN
                    if w < 2:
                        xw = xw_pre[w]
                    else:
                        xw = em_xw(w)
                    if w == 1:
                        nc.sync.dma_start(sb_cos4k, cos4k[:, :])
                        nc.sync.dma_start(sb_sin4kn, sin4kn[:, :])
                    h1w, _ = rmsnorm_cols(p_n1, pp_12, xw, WIN, "n1")
                    for rch in range(RCH):
                        cps = pp_12.tile([P, WIN], F32, tag="mm", name="mm")
                        for dch in range(DCH):
                            nc.tensor.matmul(
                                cps, sb_wdkv[:, dch, rch * P:(rch + 1) * P],
                                h1w[dch],
                                start=(dch == 0), stop=(dch == DCH - 1))
                        nc.scalar.copy(cT[rch][:, c0:c0 + WIN], cps)
                    for tb in range(w * (WIN // P), (w + 1) * (WIN // P)):
                        for nh in range(2):
                            vps = pp_12.tile([P, 512], F32, tag="mm",
                                             name="mm")
                            for rch in range(RCH):
                                nc.tensor.matmul(
                                    vps, cT[rch][:, tb * P:(tb + 1) * P],
                                    sb_wuv[:, rch, nh * 512:(nh + 1) * 512],
                                    start=(rch == 0), stop=(rch == RCH - 1))
                            dst = vext[tb][:, :].rearrange(
                                "p (h s) -> p h s", s=HD1)
                            nc.vector.tensor_copy(
                                dst[:, nh * 8:(nh + 1) * 8, 0:HD],
                                vps[:, :].rearrange("p (h s) -> p h s", s=HD))
                        oc = vext[tb][:, :].rearrange(
                            "p (h s) -> p h s", s=HD1)[:, :, HD:HD1]
                        nc.vector.memset(oc, 1.0)

            # ---- phase C: per head pair, kT+rope then pipelined flash ----
            x2T = [p_x2.tile([P, TOK], F32, tag=f"x2T{i}", name=f"x2T{i}")
                   for i in range(DCH)]
            # group the causal blocks: equal-N tail blocks share one PSUM
            # tile and one exp activation to amortize per-op overhead
            jgroups = ([[j] for j in range(8)]
                       + [[8, 9], [10, 11], [12, 13, 14, 15]])

            with ExitStack() as s6:
                p_fl = s6.enter_context(tc.tile_pool(name="fl", bufs=2))
                p_wo2 = s6.enter_context(tc.tile_pool(name="wop", bufs=1))
                pp_fl = s6.enter_context(
                    tc.tile_pool(name="pfl", bufs=2, space="PSUM"))
                sb_mask = p_wo2.tile([P, NB, 2, P], BF16, tag="mask",
                                     name="mask")
                nc.sync.dma_start(
                    sb_mask, maskt[:, :, :, :].rearrange("j g k q -> k j g q"))
                sb_wo = p_wo2.tile([P, DCH, D], BF16, tag="wo", name="wo")
                nc.sync.dma_start(
                    sb_wo, wo[:, :].rearrange("(c p) n -> p c n", p=P))

                def em_scores(kt, hp, grp):
                    N = (SL - grp[0] // 4) * P
                    s2 = pp_fl.tile([P, 2, 512], F32, tag="s2", name="s2")
                    for gi, j in enumerate(grp):
                        o = gi * N
                        jc = slice(j * P, (j + 1) * P)
                        nc.tensor.matmul(s2[:, 0, o:o + N], kt[0:HD, jc],
                                         qTa[hp][0:HD, 0:N],
                                         start=True, stop=False,
                                         skip_group_check=True)
                        nc.tensor.matmul(s2[:, 1, o:o + N], kt[HD:P, jc],
                                         qTa[hp][HD:P, 0:N],
                                         start=True, stop=False,
                                         skip_group_check=True)
                        # additive causal mask folded into the PSUM via
                        # identity-stationary matmul (-1e9 when masked)
                        nc.tensor.matmul(s2[:, :, o + N - P:o + N],
                                         ident_bf, sb_mask[:, j, :, :],
                                         start=False, stop=True,
                                         skip_group_check=True)
                    return s2

                kt_cur = kt0
                for w in range(NW):
                    em_kt_win(kt_cur, 0, w, pp_fl, "s2", None)
                for hp in range(NHP):
                    # next head pair's kT builds *inside* this head pair's
                    # flash loop: the rope vector work hides under the
                    # scores/AV matmuls instead of bunching at the boundary
                    kt_next = (p_kt.tile([P, S], BF16, tag="kTa", name="kTa")
                               if hp + 1 < NHP else None)

                    O2 = pp_fl.tile([P, 2, 512], F32, tag="O2", name="O2")
                    s2p = em_scores(kt_cur, hp, jgroups[0])
                    for gidx, grp in enumerate(jgroups):
                        N = (SL - grp[0] // 4) * P
                        G = len(grp)
                        s2n = (em_scores(kt_cur, hp, jgroups[gidx + 1])
                               if gidx < len(jgroups) - 1 else None)
                        if kt_next is not None and gidx in (2, 4, 6, 8):
                            em_kt_win(kt_next, hp + 1, (gidx - 2) // 2,
                                      pp_fl, "s2", None)
                        e2 = p_fl.tile([P, 2, 512], BF16, tag="e2",
                                       name="e2", bufs=3)
                        nc.scalar.activation(e2[:, :, 0:G * N],
                                             s2p[:, :, 0:G * N],
                                             AF.Exp, scale=0.125)
                        for gi, j in enumerate(grp):
                            o = gi * N
                            ve = vext[j][:, :].rearrange("p (h s) -> p h s",
                                                         s=HD1)
                            nc.tensor.matmul(
                                O2[0:HD1, 0, 0:N], ve[:, hp * 2, :],
                                e2[:, 0, o:o + N], start=(j == 0),
                                stop=(j == NB - 1), skip_group_check=True)
                            nc.tensor.matmul(
                                O2[0:HD1, 1, 0:N], ve[:, hp * 2 + 1, :],
                                e2[:, 1, o:o + N], start=(j == 0),
                                stop=(j == NB - 1), skip_group_check=True)
                        s2p = s2n
                    # normalize: attnT = O[0:64] * (1/l); l sits in row 64
                    sums = p_fl.tile([1, 2, TOK], F32, tag="sums",
                                     name="sums")
                    nc.scalar.copy(sums, O2[HD:HD1, :, 0:TOK])
                    sw = p_fl.tile([P, 2 * TOK // P], F32, tag="sw",
                                   name="sw")
                    nc.sync.dma_start(sw, sums)
                    rw = p_fl.tile([P, 2 * TOK // P], F32, tag="rw",
                                   name="rw")
                    nc.vector.reciprocal(rw, sw)
                    linv = p_fl.tile([1, 2, TOK], F32, tag="linv",
                                     name="linv")
                    nc.sync.dma_start(linv, rw)
                    if hp < NHP - 1:
                        # broadcast off the tensor engine (it's saturated)
                        lps = p_fl.tile([HD, 2, TOK], F32, tag="lps",
                                        name="lps")
                        nc.gpsimd.partition_broadcast(lps, linv)
                    else:
                        # last head pair: the tensor engine is idle here and
                        # the matmul broadcast is bit-identical
                        lpp = pp_fl.tile([P, 2, 512], F32, tag="s2",
                                         name="s2")
                        for g in range(2):
                            nc.tensor.matmul(lpp[0:HD, g, 0:TOK], ones_row,
                                             linv[0:1, g, :],
                                             start=True, stop=True)
                        lps = lpp[0:HD, :, 0:TOK]
                    onum = p_fl.tile([HD, 2, TOK], BF16, tag="onum",
                                     name="onum")
                    nc.scalar.copy(onum, O2[0:HD, :, 0:TOK])
                    nc.vector.tensor_tensor(attnT[hp][0:HD, :], onum[:, 0, :],
                                            lps[:, 0, :], ALU.mult)
                    a2 = p_fl.tile([HD, TOK], BF16, tag="a2", name="a2")
                    nc.vector.tensor_tensor(a2, onum[:, 1, :],
                                            lps[:, 1, :], ALU.mult)
                    # head 2 belongs on partitions 64:128 -> move via DMA
                    nc.sync.dma_start(attnT[hp][HD:P, :], a2)
                    kt_cur = kt_next

                # wo projection + residual -> x2T
                for dch in range(DCH):
                    yps = pp_fl.tile([P, 2, 512], F32, tag="s2", name="s2")
                    for hch in range(DCH):
                        nc.tensor.matmul(
                            yps[:, 0, :],
                            sb_wo[:, hch, dch * P:(dch + 1) * P],
                            attnT[hch],
                            start=(hch == 0), stop=(hch == DCH - 1))
                    nc.vector.tensor_tensor(x2T[dch], yps[:, 0, :],
                                            sb_xq[dch], ALU.add)

        # ================= router + MoE =================
        with ExitStack() as mctx:
            p_moe = mctx.enter_context(tc.tile_pool(name="moe", bufs=1))
            p_sm = mctx.enter_context(tc.tile_pool(name="sm", bufs=2))

            sb_wrn = p_moe.tile([P, DCH, E], F32, tag="wrn", name="wrn")
            nc.sync.dma_start(sb_wrn,
                              wrn[:, :].rearrange("(c p) e -> p c e", p=P))
            sb_bias = p_moe.tile([P, E], F32, tag="bias", name="bias")
            nc.sync.dma_start(sb_bias, bias128[:, :])
            gatesT = p_moe.tile([E, TOK], BF16, tag="gatesT", name="gatesT")
            x8 = [p_moe.tile([P, 2, TOK], FP8, tag=f"x8_{c}", name=f"x8_{c}")
                  for c in range(DCH // 2)]
            # prefetch the expert weights: w2 (all 8) and the first w13s
            # stream in during the router computation
            p_mw = mctx.enter_context(tc.tile_pool(name="mw", bufs=3))
            p_w2 = mctx.enter_context(tc.tile_pool(name="w2p", bufs=1))
            w2all = []
            for e in range(E):
                t = p_w2.tile([P, 2, D], FP8, tag=f"w2_{e}", name=f"w2_{e}")
                nc.sync.dma_start(t, w2dr[e, :, :, :])
                w2all.append(t)
            w13all = []

            def em_w13(e):
                t = p_mw.tile([P, DCH // 2, 2, 2 * MH], FP8,
                              tag="w13t", name="w13t")
                nc.sync.dma_start(t, w13dr[e, :, :, :, :].rearrange(
                    "c p j n -> p c j n"))
                w13all.append(t)

            # only as many prefetches as there are buffers: a deeper queue
            # would block the sync engine on buffer recycling
            for e in range(3):
                em_w13(e)
            with ExitStack() as rctx:
                pp_r = rctx.enter_context(
                    tc.tile_pool(name="pr", bufs=2, space="PSUM"))
                # rmsnorm of x2 (fp32 path kept exact for the router)
                sq2 = []
                for dch in range(DCH):
                    t = p_sm.tile([P, TOK], BF16, tag=f"sq{dch % 2}",
                                  name=f"sq{dch % 2}")
                    nc.scalar.activation(t, x2T[dch], AF.Square)
                    sq2.append(t)
                ss2 = pp_r.tile([P, TOK], F32, tag="ss", name="ss", bufs=1)
                for dch in range(DCH):
                    nc.tensor.matmul(ss2[0:1, :], ones_bf, sq2[dch],
                                     start=(dch == 0), stop=(dch == DCH - 1))
                sd2 = p_sm.tile([1, TOK], F32, tag="sd", name="sd")
                nc.scalar.activation(sd2, ss2[0:1, :], AF.Sqrt,
                                     bias=eps1, scale=1.0 / D)
                sdw2 = p_sm.tile([P, TOK // P], F32, tag="sdw", name="sdw")
                nc.sync.dma_start(sdw2, sd2)
                rcw2 = p_sm.tile([P, TOK // P], F32, tag="rcw", name="rcw")
                nc.vector.reciprocal(rcw2, sdw2)
                rsv2 = p_sm.tile([1, TOK], F32, tag="rsv", name="rsv")
                nc.sync.dma_start(rsv2, rcw2)
                rsb2 = p_sm.tile([P, TOK], F32, tag="rsb2", name="rsb2",
                                 bufs=1)
                nc.gpsimd.partition_broadcast(rsb2, rsv2)
                x2nf = []
                for dch in range(DCH):
                    t = p_moe.tile([P, TOK], F32, tag=f"x2nf{dch}",
                                   name=f"x2nf{dch}")
                    nc.vector.tensor_tensor(t, x2T[dch], rsb2, ALU.mult)
                    x2nf.append(t)
                # fp8 copy of the normalized activations for the experts
                for c in range(DCH // 2):
                    for jj in range(2):
                        nc.vector.tensor_scalar(x8[c][:, jj, :],
                                                x2nf[2 * c + jj],
                                                S_X, None, ALU.mult)
                # router scores in [E, TOK] layout: wr stationary
                scps = pp_r.tile([E, TOK], F32, tag="scp", name="scp",
                                 bufs=1)
                for dch in range(DCH):
                    nc.tensor.matmul(scps, sb_wrn[:, dch, :], x2nf[dch],
                                     start=(dch == 0), stop=(dch == DCH - 1))
                scs = p_sm.tile([E, TOK], F32, tag="scs", name="scs")
                nc.scalar.copy(scs, scps)
                tps = pp_r.tile([P, SL, E], F32, tag="tps", name="tps",
                                bufs=1)
                for c in range(SL):
                    nc.tensor.transpose(tps[:, c, :],
                                        scs[0:E, c * P:(c + 1) * P],
                                        sb_ident[0:E, 0:E])
                # gating math vectorized over all SL blocks at once
                sg = p_sm.tile([P, SL, E], F32, tag="sg", name="sg")
                nc.scalar.activation(sg, tps, AF.Sigmoid)
                tt = p_sm.tile([P, SL, E], F32, tag="tt", name="tt")
                nc.vector.tensor_tensor(
                    tt, sg,
                    sb_bias[:, :].unsqueeze(1).broadcast_to([P, SL, E]),
                    ALU.add)
                m1 = p_sm.tile([P, SL, 1], F32, tag="m1", name="m1")
                nc.vector.tensor_reduce(m1, tt, mybir.AxisListType.X,
                                        ALU.max)
                e1 = p_sm.tile([P, SL, E], F32, tag="e1", name="e1")
                nc.vector.tensor_tensor(e1, tt,
                                        m1.broadcast_to([P, SL, E]),
                                        ALU.is_ge)
                pen = p_sm.tile([P, SL, E], F32, tag="pen", name="pen")
                nc.vector.tensor_scalar(pen, e1, -1e9, None, ALU.mult)
                t2 = p_sm.tile([P, SL, E], F32, tag="t2", name="t2")
                nc.vector.tensor_tensor(t2, tt, pen, ALU.add)
                m2 = p_sm.tile([P, SL, 1], F32, tag="m2", name="m2")
                nc.vector.tensor_reduce(m2, t2, mybir.AxisListType.X,
                                        ALU.max)
                e2g = p_sm.tile([P, SL, E], F32, tag="e2g", name="e2g")
                nc.vector.tensor_tensor(e2g, t2,
                                        m2.broadcast_to([P, SL, E]),
                                        ALU.is_ge)
                sel = p_sm.tile([P, SL, E], F32, tag="sel", name="sel")
                nc.vector.tensor_tensor(sel, e1, e2g, ALU.add)
                gg = p_sm.tile([P, SL, E], F32, tag="gg", name="gg")
                nc.vector.tensor_tensor(gg, sg, sel, ALU.mult)
                dsum = p_sm.tile([P, SL, 1], F32, tag="dsum", name="dsum")
                nc.vector.tensor_reduce(dsum, gg, mybir.AxisListType.X,
                                        ALU.add)
                nc.vector.tensor_scalar(dsum, dsum, 1e-9, None, ALU.add)
                rcp = p_sm.tile([P, SL, 1], F32, tag="rcp", name="rcp")
                nc.vector.reciprocal(rcp, dsum)
                nc.vector.tensor_tensor(gg, gg,
                                        rcp.broadcast_to([P, SL, E]),
                                        ALU.mult)
                for c in range(SL):
                    gtp = pp_r.tile([E, P], F32, tag="gtp", name="gtp",
                                    bufs=2)
                    nc.tensor.transpose(gtp, gg[:, c, :], sb_ident)
                    nc.scalar.activation(gatesT[:, c * P:(c + 1) * P], gtp,
                                         AF.Copy, scale=C_GB)

            h2g = [p_moe.tile([P, 2, TOK], FP8, tag=f"h2g{e}",
                              name=f"h2g{e}") for e in range(E)]
            with ExitStack() as ectx:
                pp_hps = ectx.enter_context(
                    tc.tile_pool(name="phps", bufs=1, space="PSUM"))
                for e in range(E):
                    w13t = w13all[e]
                    if e + 3 < E:
                        em_w13(e + 3)
                    # hop the gate row to partition 0 first: the gpsimd
                    # broadcast reads partition 0 of its input AP
                    g1 = p_mw.tile([1, TOK], BF16, tag="g1", name="g1")
                    nc.sync.dma_start(g1, gatesT[e:e + 1, :])
                    gb = p_mw.tile([P, TOK], BF16, tag="gb", name="gb")
                    nc.gpsimd.partition_broadcast(gb, g1)
                    hpre = []
                    for m in range(4):
                        hps = pp_hps.tile([P, TOK], F32, tag=f"hps{m}",
                                          name=f"hps{m}")
                        for c in range(DCH // 2):
                            nc.tensor.matmul(
                                hps, w13t[:, c, :, m * P:(m + 1) * P],
                                x8[c],
                                start=(c == 0), stop=(c == DCH // 2 - 1),
                                perf_mode=mybir.MatmulPerfMode.DoubleRow)
                        hpre.append(hps)
                    for mc in range(2):
                        sa = p_sm.tile([P, TOK], BF16, tag="sa", name="sa")
                        nc.scalar.activation(sa, hpre[mc], AF.Silu,
                                             scale=C_SILU)
                        tg = p_sm.tile([P, TOK], BF16, tag="tg", name="tg")
                        nc.vector.tensor_tensor(tg, sa, hpre[mc + 2],
                                                ALU.mult)
                        nc.vector.tensor_tensor(h2g[e][:, mc, :], tg, gb,
                                                ALU.mult)

                with ExitStack() as w2ctx:
                    pp_yf = w2ctx.enter_context(
                        tc.tile_pool(name="pyf", bufs=2, space="PSUM"))
                    for dch in range(DCH):
                        yf = pp_yf.tile([P, TOK], F32, tag="yf", name="yf")
                        for e in range(E):
                            nc.tensor.matmul(
                                yf, w2all[e][:, :, dch * P:(dch + 1) * P],
                                h2g[e],
                                start=(e == 0), stop=(e == E - 1),
                                perf_mode=mybir.MatmulPerfMode.DoubleRow)
                        ot = p_sm.tile([P, TOK], F32, tag="ot", name="ot")
                        nc.vector.scalar_tensor_tensor(
                            ot, yf, C_OUT, x2T[dch], ALU.mult, ALU.add)
                        nc.sync.dma_start(outT[dch * P:(dch + 1) * P, :], ot)

    nc.compile()
    return nc


_NC_CACHE = {}


def _get_nc(S):
    if S not in _NC_CACHE:
        _NC_CACHE[S] = _build(S)
    return _NC_CACHE[S]


def _q8(v, s):
    return np.clip(v * s, -240.0, 240.0).astype(F8)


def host_prep(x, position_ids, norm1_w, wq, wdkv, wuk, wuv, wo,
              norm2_w, wr, router_bias, w1, w3, w2):
    x = np.asarray(x, np.float32)
    _, S, _ = x.shape
    NB = S // P
    SL = NB // 4

    pos = np.asarray(position_ids, np.int32)
    norm1_w = np.asarray(norm1_w, np.float32)
    norm2_w = np.asarray(norm2_w, np.float32)
    wq_n = (np.asarray(wq, np.float32) * norm1_w[:, None]).astype(BF)
    wdkv_n = (np.asarray(wdkv, np.float32) * norm1_w[:, None]).astype(BF)
    wuk_f = np.asarray(wuk, np.float32)
    # [R, 2, H*HD]: straight copy and per-head half-swapped copy so the
    # rope rotation needs no on-chip partition moves
    perm = np.concatenate([np.arange(h * HD + HALF, h * HD + HD).tolist() +
                           np.arange(h * HD, h * HD + HALF).tolist()
                           for h in range(H)]).astype(np.int64)
    wukx_b = np.ascontiguousarray(
        np.stack([wuk_f, wuk_f[:, perm]], axis=1)).astype(BF)
    wuv_b = np.asarray(wuv, np.float32).astype(BF)
    wo_b = np.asarray(wo, np.float32).astype(BF)
    wr_n = np.ascontiguousarray(np.asarray(wr, np.float32) * norm2_w[:, None])
    w13 = np.concatenate([np.asarray(w1, np.float32),
                          np.asarray(w3, np.float32)], axis=2)
    w13_n = w13 * norm2_w[None, :, None]
    # DoubleRow interleave: contract chunk c pairs d = c*256 + j*128 + ki
    w13_dr = np.ascontiguousarray(
        _q8(w13_n.reshape(E, 4, 2, P, 2 * MH).transpose(0, 1, 3, 2, 4),
            S_W13))
    w2_dr = np.ascontiguousarray(
        _q8(np.asarray(w2, np.float32).reshape(E, 2, P, D).transpose(
            0, 2, 1, 3), S_W2))
    bias_b = np.ascontiguousarray(np.broadcast_to(
        np.asarray(router_bias, np.float32)[None, :], (P, E)))
    ident = np.eye(P, dtype=np.float32)

    inv = 1.0 / (THETA ** (np.arange(HALF, dtype=np.float64) / HALF))

    in_maps = []
    slot_blocks_all = []
    for c in range(NCORES):
        b, r = divmod(c, 4)
        slot_blocks = [r + 4 * (SL - 1 - m) for m in range(SL)]
        slot_blocks_all.append(slot_blocks)
        own = np.concatenate(
            [np.arange(g * P, (g + 1) * P) for g in slot_blocks])

        ang = pos[b].astype(np.float64)[:, None] * inv[None, :]
        cosT = np.cos(ang).T.astype(np.float32)
        sinT = np.sin(ang).T.astype(np.float32)
        cos4k_h = np.tile(cosT, (4, 1)).astype(BF)
        sin4kn_h = np.concatenate([-sinT, sinT, -sinT, sinT], 0).astype(BF)
        cos4q_h = np.ascontiguousarray(cos4k_h[:, own])
        sin4qn_h = np.ascontiguousarray(sin4kn_h[:, own])

        xT_h = np.ascontiguousarray(x[b].T)
        xTq_h = np.ascontiguousarray(x[b].T[:, own])

        # additive mask: 0 where attention is allowed, -1e9 where masked
        maskt_h = np.full((NB, P, P), -1e9, np.float32)
        for j in range(NB):
            jm = j % 4
            if jm < r:
                maskt_h[j] = 0.0
            elif jm == r:
                maskt_h[j] = (np.triu(np.ones((P, P), np.float32))
                              - 1.0) * 1e9
        maskt_h = np.repeat(maskt_h[:, None, :, :], 2, axis=1).astype(BF)

        in_maps.append({
            "xT": xT_h, "xTq": xTq_h,
            "cos4k": cos4k_h, "sin4kn": sin4kn_h,
            "cos4q": cos4q_h, "sin4qn": sin4qn_h,
            "maskt": maskt_h,
            "wqn": wq_n, "wdkvn": wdkv_n, "wukx": wukx_b, "wuv": wuv_b,
            "wo": wo_b, "wrn": wr_n, "bias128": bias_b,
            "w13dr": w13_dr, "w2dr": w2_dr, "identf": ident,
        })
    return in_maps, slot_blocks_all


def run(inputs, trace=False):
    x = np.asarray(inputs["x"], np.float32)
    Bx, S, Dx = x.shape
    nc = _get_nc(S)
    in_maps, slot_blocks_all = host_prep(**inputs)
    res = run_bass_kernel_spmd(nc, in_maps, core_ids=list(range(NCORES)),
                               trace=trace)
    out = np.zeros((Bx, S, Dx), np.float32)
    for c in range(NCORES):
        b = c // 4
        oT = np.asarray(res.results[c]["outT"])
        for m, g in enumerate(slot_blocks_all[c]):
            out[b, g * P:(g + 1) * P, :] = oT[:, m * P:(m + 1) * P].T
    return out, res


def kernel(**inputs):
    out, _ = run(inputs)
    return out


# revision 74
# speedup vs baseline: 1.0213x; 1.0213x over previous
"""DeepSeekV3 block (MLA attention + top-2-of-8 MoE) on 8 trn2 NeuronCores.

Sharding: cores 0-3 -> batch 0, cores 4-7 -> batch 1. Within a batch group
of 4 cores, each core owns S/4 query tokens chosen as SL strided 128-row
blocks ordered by causal depth (blocks r+12, r+8, r+4, r for sub-rank r at
S=2048), which makes the flash-attention k-loop narrow uniformly across
cores: one SPMD program, all per-core differences live in input data.
k/v/latent projections are recomputed per core (replicated within the
batch group) to avoid collectives.

v2 changes vs baseline:
- q projection runs first (overlaps the xT window DMAs); window loop is
  double-buffered and fuses the per-window v up-projection.
- flash attention is software-pipelined: scores for block j+1 are issued
  ahead of the AV matmuls for block j so the in-order tensor queue never
  stalls on the softmax exp; the epilogue reciprocal runs directly on the
  PSUM row (no DMA round trip) and O2 is double-banked so the next head
  pair starts immediately.
- router matmul is reoriented (small wr stationary, tokens streamed) with
  the gating math vectorized over all four 128-token blocks at once.
- MoE expert matmuls run in fp8 (e4m3) with DoubleRow packing: weights are
  pre-scaled/interleaved on the host, activations are quantized on-chip,
  and all scale factors fold into existing activation/broadcast ops.
  Gate values are folded into the hidden states via a gpsimd partition
  broadcast instead of a tensor-engine broadcast matmul.

Layout convention: activations are kept transposed [feature, token] so
weight matrices are always the stationary matmul operand, and softmax
denominators come from a ones column appended to the value tiles.
"""

import sys

sys.path.insert(0, "/opt/trn_rl_repo")

from contextlib import ExitStack

import ml_dtypes
import numpy as np

import concourse.bass as bass
import concourse.tile as tile
from concourse import bacc
from concourse import mybir
from concourse.bass_utils import run_bass_kernel_spmd

F32 = mybir.dt.float32
BF16 = mybir.dt.bfloat16
FP8 = mybir.dt.float8e4
AF = mybir.ActivationFunctionType
ALU = mybir.AluOpType
BF = ml_dtypes.bfloat16
F8 = ml_dtypes.float8_e4m3

B, D = 2, 1024
H, HD = 16, 64
HALF = HD // 2
R = 256
E, TOPK, MH = 8, 2, 256
EPS = 1e-6
THETA = 10000.0
P = 128
NCORES = 8

# fp8 scale plan for the MoE: every scale is a power of two so the
# compensations fold exactly into activation scales.
S_X = 32.0        # x2n -> fp8
S_W13 = 2048.0    # w1/w3 -> fp8
S_W2 = 2048.0     # w2 -> fp8
S_H = 8.0         # hidden*gate -> fp8
C_SILU = 1.0 / (S_X * S_W13)          # descale inside the silu activation
C_GB = S_H / (S_X * S_W13)            # folded into the gate broadcast
C_OUT = 1.0 / (S_H * S_W2)            # final descale before the residual


def _build(S: int):
    NB = S // P               # seq blocks per batch (16 at S=2048)
    SL = NB // 4              # q-block slots per core
    TOK = SL * P              # own tokens per core
    WIN = min(512, S)
    NW = S // WIN
    NHP = H // 2              # 8 head pairs
    DCH = D // P              # 8
    RCH = R // P              # 2
    HD1 = HD + 1

    nc = bacc.Bacc(None, target_bir_lowering=False)

    xT = nc.dram_tensor("xT", [D, S], F32, kind="ExternalInput")
    xTq = nc.dram_tensor("xTq", [D, TOK], F32, kind="ExternalInput")
    cos4k = nc.dram_tensor("cos4k", [P, S], BF16, kind="ExternalInput")
    sin4kn = nc.dram_tensor("sin4kn", [P, S], BF16, kind="ExternalInput")
    cos4q = nc.dram_tensor("cos4q", [P, TOK], BF16, kind="ExternalInput")
    sin4qn = nc.dram_tensor("sin4qn", [P, TOK], BF16, kind="ExternalInput")
    maskt = nc.dram_tensor("maskt", [NB, 2, P, P], BF16, kind="ExternalInput")
    wqn = nc.dram_tensor("wqn", [D, H * HD], BF16, kind="ExternalInput")
    wdkvn = nc.dram_tensor("wdkvn", [D, R], BF16, kind="ExternalInput")
    wukx = nc.dram_tensor("wukx", [R, 2, H * HD], BF16, kind="ExternalInput")
    wuv = nc.dram_tensor("wuv", [R, H * HD], BF16, kind="ExternalInput")
    wo = nc.dram_tensor("wo", [H * HD, D], BF16, kind="ExternalInput")
    wrn = nc.dram_tensor("wrn", [D, E], F32, kind="ExternalInput")
    bias128 = nc.dram_tensor("bias128", [P, E], F32, kind="ExternalInput")
    w13dr = nc.dram_tensor("w13dr", [E, DCH // 2, P, 2, 2 * MH], FP8,
                           kind="ExternalInput")
    w2dr = nc.dram_tensor("w2dr", [E, P, 2, D], FP8, kind="ExternalInput")
    identf = nc.dram_tensor("identf", [P, P], F32, kind="ExternalInput")
    outT = nc.dram_tensor("outT", [D, TOK], F32, kind="ExternalOutput")

    with tile.TileContext(nc) as tc, ExitStack() as ctx:
        p_const = ctx.enter_context(tc.tile_pool(name="const", bufs=1))
        p_x2 = ctx.enter_context(tc.tile_pool(name="x2", bufs=1))

        ones_bf = p_const.tile([P, 1], BF16, tag="ones_bf", name="ones_bf")
        nc.vector.memset(ones_bf, 1.0)
        eps1 = p_const.tile([1, 1], F32, tag="eps1", name="eps1")
        nc.vector.memset(eps1, EPS)
        sb_ident = p_const.tile([P, P], F32, tag="ident", name="ident")
        nc.sync.dma_start(sb_ident, identf[:, :])
        ident_bf = p_const.tile([P, P], BF16, tag="identb", name="identb")
        nc.scalar.copy(ident_bf, sb_ident)

        # own-token x loads first: the q projection consumes them and runs
        # while the full-sequence windows stream in behind.
        sb_xq = []
        for dch in range(DCH):
            t = p_x2.tile([P, TOK], F32, tag=f"xq{dch}", name=f"xq{dch}")
            nc.sync.dma_start(t, xTq[dch * P:(dch + 1) * P, :])
            sb_xq.append(t)

        # all big loads share the sync queue in explicit first-use order;
        # tiles are declared here, the DMAs are interleaved below
        p_wk = ctx.enter_context(tc.tile_pool(name="wk", bufs=1))
        sb_wdkv = p_wk.tile([P, DCH, R], BF16, tag="wdkv", name="wdkv")
        sb_wuv = p_wk.tile([P, RCH, H * HD], BF16, tag="wuv", name="wuv")
        sb_wuk = p_wk.tile([P, RCH, 2, H * HD], BF16, tag="wuk", name="wuk")
        sb_cos4k = p_wk.tile([P, S], BF16, tag="cos4k", name="cos4k")
        sb_sin4kn = p_wk.tile([P, S], BF16, tag="sin4kn", name="sin4kn")

        def rmsnorm_cols(pool, ppool, src_tiles, ncols, nametag):
            """src_tiles: DCH sbuf [P, ncols] f32 -> DCH bf16 tiles,
            rms-normalized across the full d axis. Also returns the psum
            broadcast of 1/rms for fp32 consumers."""
            sq = []
            for dch in range(DCH):
                t = pool.tile([P, ncols], BF16, tag=f"sq{dch % 2}",
                              name=f"sq{dch % 2}")
                nc.scalar.activation(t, src_tiles[dch], AF.Square)
                sq.append(t)
            ss = ppool.tile([P, ncols], F32, tag="ss", name="ss")
            for dch in range(DCH):
                nc.tensor.matmul(ss[0:1, :], ones_bf, sq[dch],
                                 start=(dch == 0), stop=(dch == DCH - 1))
            sd = pool.tile([1, ncols], F32, tag="sd", name="sd")
            nc.scalar.activation(sd, ss[0:1, :], AF.Sqrt,
                                 bias=eps1, scale=1.0 / D)
            sdw = pool.tile([P, ncols // P], F32, tag="sdw", name="sdw")
            nc.sync.dma_start(sdw, sd)
            rcw = pool.tile([P, ncols // P], F32, tag="rcw", name="rcw")
            nc.vector.reciprocal(rcw, sdw)
            rsv = pool.tile([1, ncols], F32, tag="rsv", name="rsv")
            nc.sync.dma_start(rsv, rcw)
            rsb = pool.tile([P, ncols], F32, tag="rsb_s", name="rsb_s")
            nc.gpsimd.partition_broadcast(rsb, rsv)
            out = []
            for dch in range(DCH):
                t = pool.tile([P, ncols], BF16, tag=f"h_{nametag}{dch}",
                              name=f"h_{nametag}{dch}")
                nc.vector.tensor_tensor(t, src_tiles[dch], rsb, ALU.mult)
                out.append(t)
            return out, rsb

        def rope6(pool, pre_ps, cos_t, sin_t, out_tile, nametag):
            """rope on psum [P, cols] (2 heads stacked) -> bf16 out_tile.
            Engines are partition-lane-locked, so the half-swap goes
            through SBUF->SBUF DMA."""
            kbf = pool.tile(list(out_tile.shape), BF16, tag=f"rkb_{nametag}",
                            name=f"rkb_{nametag}")
            nc.scalar.copy(kbf, pre_ps)
            ksw = pool.tile(list(out_tile.shape), BF16, tag=f"rsw_{nametag}",
                            name=f"rsw_{nametag}")
            # gpsimd queue: keeps the partition swap off the sync queue,
            # which carries the latency-critical input loads
            for g in range(4):
                a = g * HALF
                pa = (g + 1) * HALF if g % 2 == 0 else (g - 1) * HALF
                nc.gpsimd.dma_start(ksw[a:a + HALF, :], kbf[pa:pa + HALF, :])
            tmp = pool.tile(list(out_tile.shape), BF16, tag=f"rtm_{nametag}",
                            name=f"rtm_{nametag}")
            nc.vector.tensor_tensor(tmp, ksw, sin_t, ALU.mult)
            nc.vector.tensor_tensor(out_tile, kbf, cos_t, ALU.mult)
            nc.vector.tensor_tensor(out_tile, out_tile, tmp, ALU.add)

        attnT = [p_x2.tile([P, TOK], BF16, tag=f"attnT{i}", name=f"attnT{i}")
                 for i in range(NHP)]
        qTa = [p_x2.tile([P, TOK], BF16, tag=f"qTa{i}", name=f"qTa{i}")
               for i in range(NHP)]

        with ExitStack() as kvctx:
            p_kv = kvctx.enter_context(tc.tile_pool(name="kv", bufs=1))
            vext = [p_kv.tile([P, H * HD1], BF16, tag=f"vext{i}",
                              name=f"vext{i}") for i in range(NB)]
            cT = [p_kv.tile([P, S], BF16, tag=f"cT{i}", name=f"cT{i}")
                  for i in range(RCH)]

            # ---- phase A: qT + rope (own tokens; overlaps window DMAs) ----
            with ExitStack() as s5:
                p_q = s5.enter_context(tc.tile_pool(name="q", bufs=2))
                p_wq = s5.enter_context(tc.tile_pool(name="wqp", bufs=1))
                pp_5 = s5.enter_context(
                    tc.tile_pool(name="p5", bufs=2, space="PSUM"))
                sb_wq = p_wq.tile([P, DCH, H * HD], BF16, tag="wq", name="wq")
                nc.sync.dma_start(
                    sb_wq, wqn[:, :].rearrange("(c p) n -> p c n", p=P))
                sb_cos4q = p_wq.tile([P, TOK], BF16, tag="cos4q",
                                     name="cos4q")
                nc.sync.dma_start(sb_cos4q, cos4q[:, :])
                sb_sin4qn = p_wq.tile([P, TOK], BF16, tag="sin4qn",
                                      name="sin4qn")
                nc.sync.dma_start(sb_sin4qn, sin4qn[:, :])
                h1q, _ = rmsnorm_cols(p_q, pp_5, sb_xq, TOK, "nq")
                for hp in range(NHP):
                    hc = hp * 2 * HD
                    qps = pp_5.tile([P, TOK], F32, tag="mm", name="mm")
                    for dch in range(DCH):
                        nc.tensor.matmul(
                            qps, sb_wq[:, dch, hc:hc + P], h1q[dch],
                            start=(dch == 0), stop=(dch == DCH - 1))
                    rope6(p_q, qps, sb_cos4q, sb_sin4qn, qTa[hp], "q")

            p_kt = kvctx.enter_context(tc.tile_pool(name="kt", bufs=2))

            def em_kt_win(kt, khp, w, ppool, ptag, pbufs):
                hc = khp * 2 * HD
                c0 = w * WIN
                if pbufs is None:
                    kps = ppool.tile([P, 2, 512], F32, tag=ptag, name=ptag)
                else:
                    kps = ppool.tile([P, 2, 512], F32, tag=ptag, name=ptag,
                                     bufs=pbufs)
                for rch in range(RCH):
                    nc.tensor.matmul(
                        kps[:, 0, 0:WIN],
                        sb_wuk[:, rch, 0, hc:hc + P],
                        cT[rch][:, c0:c0 + WIN],
                        start=(rch == 0), stop=(rch == RCH - 1))
                kbf2 = p_kt.tile([P, 2, WIN], BF16, tag="kbf2", name="kbf2")
                nc.vector.tensor_copy(kbf2[:, 0, :], kps[:, 0, 0:WIN])
                # rope half-swap via SBUF DMA (sync queue has slack; the
                # tensor engine is the throttled resource): bit-identical
                # to the permuted-weight matmul it replaces
                for g in range(4):
                    a = g * HALF
                    pa = (g + 1) * HALF if g % 2 == 0 else (g - 1) * HALF
                    nc.sync.dma_start(kbf2[a:a + HALF, 1, :],
                                      kbf2[pa:pa + HALF, 0, :])
                ktmp = p_kt.tile([P, WIN], BF16, tag="ktmp", name="ktmp")
                nc.vector.tensor_tensor(ktmp, kbf2[:, 1, :],
                                        sb_sin4kn[:, c0:c0 + WIN], ALU.mult)
                nc.vector.tensor_tensor(kt[:, c0:c0 + WIN], kbf2[:, 0, :],
                                        sb_cos4k[:, c0:c0 + WIN], ALU.mult)
                nc.vector.tensor_tensor(kt[:, c0:c0 + WIN],
                                        kt[:, c0:c0 + WIN], ktmp, ALU.add)

            kt0 = p_kt.tile([P, S], BF16, tag="kTa", name="kTa")

            # ---- phase B: per window rmsnorm -> latent cT -> v up;
            # head pair 0's kT is built as each window's cT lands ----
            with ExitStack() as s12:
                p_xw = s12.enter_context(tc.tile_pool(name="xw", bufs=2))
                p_n1 = s12.enter_context(tc.tile_pool(name="n1", bufs=2))
                pp_12 = s12.enter_context(
                    tc.tile_pool(name="p12", bufs=2, space="PSUM"))

                def em_xw(w):
                    c0 = w * WIN
                    xw = []
                    for dch in range(DCH):
                        t = p_xw.tile([P, WIN], F32, tag=f"xw{dch}",
                                      name=f"xw{dch}")
                        nc.sync.dma_start(
                            t, xT[dch * P:(dch + 1) * P, c0:c0 + WIN])
                        xw.append(t)
                    return xw

                # windows 0/1 load right behind the phase-A inputs; the
                # flash-phase weights are interleaved between the remaining
                # windows so everything lands just before first use
                xw_pre = [em_xw(0), em_xw(1)]
                nc.sync.dma_start(
                    sb_wdkv, wdkvn[:, :].rearrange("(c p) r -> p c r", p=P))
                nc.sync.dma_start(
                    sb_wuv, wuv[:, :].rearrange("(c p) n -> p c n", p=P))
                nc.sync.dma_start(
                    sb_wuk,
                    wukx[:, :, :].rearrange("(c p) s n -> p c s n", p=P))
                for w in range(NW):
                    c0 = w * WIN
                    if w < 2:
                        xw = xw_pre[w]
                    else:
                        xw = em_xw(w)
                    if w == 1:
                        nc.sync.dma_start(sb_cos4k, cos4k[:, :])
                        nc.sync.dma_start(sb_sin4kn, sin4kn[:, :])
                    h1w, _ = rmsnorm_cols(p_n1, pp_12, xw, WIN, "n1")
                    for rch in range(RCH):
                        cps = pp_12.tile([P, WIN], F32, tag="mm", name="mm")
                        for dch in range(DCH):
                            nc.tensor.matmul(
                                cps, sb_wdkv[:, dch, rch * P:(rch + 1) * P],
                                h1w[dch],
                                start=(dch == 0), stop=(dch == DCH - 1))
                        nc.scalar.copy(cT[rch][:, c0:c0 + WIN], cps)
                    for tb in range(w * (WIN // P), (w + 1) * (WIN // P)):
                        for nh in range(2):
                            vps = pp_12.tile([P, 512], F32, tag="mm",
                                             name="mm")
                            for rch in range(RCH):
                                nc.tensor.matmul(
                                    vps, cT[rch][:, tb * P:(tb + 1) * P],
                                    sb_wuv[:, rch, nh * 512:(nh + 1) * 512],
                                    start=(rch == 0), stop=(rch == RCH - 1))
                            dst = vext[tb][:, :].rearrange(
                                "p (h s) -> p h s", s=HD1)
                            nc.scalar.copy(
                                dst[:, nh * 8:(nh + 1) * 8, 0:HD],
                                vps[:, :].rearrange("p (h s) -> p h s", s=HD))
                        oc = vext[tb][:, :].rearrange(
                            "p (h s) -> p h s", s=HD1)[:, :, HD:HD1]
                        nc.vector.memset(oc, 1.0)

            # ---- phase C: per head pair, kT+rope then pipelined flash ----
            x2T = [p_x2.tile([P, TOK], F32, tag=f"x2T{i}", name=f"x2T{i}")
                   for i in range(DCH)]
            # group the causal blocks: equal-N tail blocks share one PSUM
            # tile and one exp activation to amortize per-op overhead
            jgroups = ([[j] for j in range(8)]
                       + [[8, 9], [10, 11], [12, 13, 14, 15]])

            with ExitStack() as s6:
                p_fl = s6.enter_context(tc.tile_pool(name="fl", bufs=2))
                p_wo2 = s6.enter_context(tc.tile_pool(name="wop", bufs=1))
                pp_fl = s6.enter_context(
                    tc.tile_pool(name="pfl", bufs=2, space="PSUM"))
                sb_mask = p_wo2.tile([P, NB, 2, P], BF16, tag="mask",
                                     name="mask")
                nc.sync.dma_start(
                    sb_mask, maskt[:, :, :, :].rearrange("j g k q -> k j g q"))
                sb_wo = p_wo2.tile([P, DCH, D], BF16, tag="wo", name="wo")
                nc.sync.dma_start(
                    sb_wo, wo[:, :].rearrange("(c p) n -> p c n", p=P))

                def em_scores(kt, hp, grp):
                    N = (SL - grp[0] // 4) * P
                    s2 = pp_fl.tile([P, 2, 512], F32, tag="s2", name="s2")
                    for gi, j in enumerate(grp):
                        o = gi * N
                        jc = slice(j * P, (j + 1) * P)
                        nc.tensor.matmul(s2[:, 0, o:o + N], kt[0:HD, jc],
                                         qTa[hp][0:HD, 0:N],
                                         start=True, stop=False,
                                         skip_group_check=True)
                        nc.tensor.matmul(s2[:, 1, o:o + N], kt[HD:P, jc],
                                         qTa[hp][HD:P, 0:N],
                                         start=True, stop=False,
                                         skip_group_check=True)
                        # additive causal mask folded into the PSUM via
                        # identity-stationary matmul (-1e9 when masked)
                        nc.tensor.matmul(s2[:, :, o + N - P:o + N],
                                         ident_bf, sb_mask[:, j, :, :],
                                         start=False, stop=True,
                                         skip_group_check=True)
                    return s2

                kt_cur = kt0
                for w in range(NW):
                    em_kt_win(kt_cur, 0, w, pp_fl, "s2", None)
                for hp in range(NHP):
                    # next head pair's kT builds *inside* this head pair's
                    # flash loop: the rope vector work hides under the
                    # scores/AV matmuls instead of bunching at the boundary
                    kt_next = (p_kt.tile([P, S], BF16, tag="kTa", name="kTa")
                               if hp + 1 < NHP else None)

                    O2 = pp_fl.tile([P, 2, 512], F32, tag="O2", name="O2")
                    s2p = em_scores(kt_cur, hp, jgroups[0])
                    for gidx, grp in enumerate(jgroups):
                        N = (SL - grp[0] // 4) * P
                        G = len(grp)
                        s2n = (em_scores(kt_cur, hp, jgroups[gidx + 1])
                               if gidx < len(jgroups) - 1 else None)
                        if kt_next is not None and gidx in (2, 4, 6, 8):
                            em_kt_win(kt_next, hp + 1, (gidx - 2) // 2,
                                      pp_fl, "s2", None)
                        e2 = p_fl.tile([P, 2, 512], BF16, tag="e2",
                                       name="e2", bufs=3)
                        nc.scalar.activation(e2[:, :, 0:G * N],
                                             s2p[:, :, 0:G * N],
                                             AF.Exp, scale=0.125)
                        for gi, j in enumerate(grp):
                            o = gi * N
                            ve = vext[j][:, :].rearrange("p (h s) -> p h s",
                                                         s=HD1)
                            nc.tensor.matmul(
                                O2[0:HD1, 0, 0:N], ve[:, hp * 2, :],
                                e2[:, 0, o:o + N], start=(j == 0),
                                stop=(j == NB - 1), skip_group_check=True)
                            nc.tensor.matmul(
                                O2[0:HD1, 1, 0:N], ve[:, hp * 2 + 1, :],
                                e2[:, 1, o:o + N], start=(j == 0),
                                stop=(j == NB - 1), skip_group_check=True)
                        s2p = s2n
                    # normalize: attnT = O[0:64] * (1/l); l sits in row 64
                    sums = p_fl.tile([1, 2, TOK], F32, tag="sums",
                                     name="sums")
                    nc.scalar.copy(sums, O2[HD:HD1, :, 0:TOK])
                    sw = p_fl.tile([P, 2 * TOK // P], F32, tag="sw",
                                   name="sw")
                    nc.sync.dma_start(sw, sums)
                    rw = p_fl.tile([P, 2 * TOK // P], F32, tag="rw",
                                   name="rw")
                    nc.vector.reciprocal(rw, sw)
                    linv = p_fl.tile([1, 2, TOK], F32, tag="linv",
                                     name="linv")
                    nc.sync.dma_start(linv, rw)
                    lps = p_fl.tile([HD, 2, TOK], F32, tag="lps", name="lps")
                    nc.gpsimd.partition_broadcast(lps, linv)
                    onum = p_fl.tile([HD, 2, TOK], BF16, tag="onum",
                                     name="onum")
                    nc.scalar.copy(onum, O2[0:HD, :, 0:TOK])
                    nc.vector.tensor_tensor(attnT[hp][0:HD, :], onum[:, 0, :],
                                            lps[:, 0, :], ALU.mult)
                    a2 = p_fl.tile([HD, TOK], BF16, tag="a2", name="a2")
                    nc.vector.tensor_tensor(a2, onum[:, 1, :],
                                            lps[:, 1, :], ALU.mult)
                    # head 2 belongs on partitions 64:128 -> move via DMA
                    nc.sync.dma_start(attnT[hp][HD:P, :], a2)
                    kt_cur = kt_next

                # wo projection + residual -> x2T
                for dch in range(DCH):
                    yps = pp_fl.tile([P, 2, 512], F32, tag="s2", name="s2")
                    for hch in range(DCH):
                        nc.tensor.matmul(
                            yps[:, 0, :],
                            sb_wo[:, hch, dch * P:(dch + 1) * P],
                            attnT[hch],
                            start=(hch == 0), stop=(hch == DCH - 1))
                    nc.vector.tensor_tensor(x2T[dch], yps[:, 0, :],
                                            sb_xq[dch], ALU.add)

        # ================= router + MoE =================
        with ExitStack() as mctx:
            p_moe = mctx.enter_context(tc.tile_pool(name="moe", bufs=1))
            p_sm = mctx.enter_context(tc.tile_pool(name="sm", bufs=2))

            sb_wrn = p_moe.tile([P, DCH, E], F32, tag="wrn", name="wrn")
            nc.sync.dma_start(sb_wrn,
                              wrn[:, :].rearrange("(c p) e -> p c e", p=P))
            sb_bias = p_moe.tile([P, E], F32, tag="bias", name="bias")
            nc.sync.dma_start(sb_bias, bias128[:, :])
            gatesT = p_moe.tile([E, TOK], BF16, tag="gatesT", name="gatesT")
            x8 = [p_moe.tile([P, 2, TOK], FP8, tag=f"x8_{c}", name=f"x8_{c}")
                  for c in range(DCH // 2)]
            # prefetch the expert weights: w2 (all 8) and the first w13s
            # stream in during the router computation
            p_mw = mctx.enter_context(tc.tile_pool(name="mw", bufs=3))
            p_w2 = mctx.enter_context(tc.tile_pool(name="w2p", bufs=1))
            w2all = []
            for e in range(E):
                t = p_w2.tile([P, 2, D], FP8, tag=f"w2_{e}", name=f"w2_{e}")
                nc.sync.dma_start(t, w2dr[e, :, :, :])
                w2all.append(t)
            w13all = []

            def em_w13(e):
                t = p_mw.tile([P, DCH // 2, 2, 2 * MH], FP8,
                              tag="w13t", name="w13t")
                nc.sync.dma_start(t, w13dr[e, :, :, :, :].rearrange(
                    "c p j n -> p c j n"))
                w13all.append(t)

            # only as many prefetches as there are buffers: a deeper queue
            # would block the sync engine on buffer recycling
            for e in range(3):
                em_w13(e)
            with ExitStack() as rctx:
                pp_r = rctx.enter_context(
                    tc.tile_pool(name="pr", bufs=2, space="PSUM"))
                # rmsnorm of x2 (fp32 path kept exact for the router)
                sq2 = []
                for dch in range(DCH):
                    t = p_sm.tile([P, TOK], BF16, tag=f"sq{dch % 2}",
                                  name=f"sq{dch % 2}")
                    nc.scalar.activation(t, x2T[dch], AF.Square)
                    sq2.append(t)
                ss2 = pp_r.tile([P, TOK], F32, tag="ss", name="ss", bufs=1)
                for dch in range(DCH):
                    nc.tensor.matmul(ss2[0:1, :], ones_bf, sq2[dch],
                                     start=(dch == 0), stop=(dch == DCH - 1))
                sd2 = p_sm.tile([1, TOK], F32, tag="sd", name="sd")
                nc.scalar.activation(sd2, ss2[0:1, :], AF.Sqrt,
                                     bias=eps1, scale=1.0 / D)
                sdw2 = p_sm.tile([P, TOK // P], F32, tag="sdw", name="sdw")
                nc.sync.dma_start(sdw2, sd2)
                rcw2 = p_sm.tile([P, TOK // P], F32, tag="rcw", name="rcw")
                nc.vector.reciprocal(rcw2, sdw2)
                rsv2 = p_sm.tile([1, TOK], F32, tag="rsv", name="rsv")
                nc.sync.dma_start(rsv2, rcw2)
                rsb2 = p_sm.tile([P, TOK], F32, tag="rsb2", name="rsb2",
                                 bufs=1)
                nc.gpsimd.partition_broadcast(rsb2, rsv2)
                x2nf = []
                for dch in range(DCH):
                    t = p_moe.tile([P, TOK], F32, tag=f"x2nf{dch}",
                                   name=f"x2nf{dch}")
                    nc.vector.tensor_tensor(t, x2T[dch], rsb2, ALU.mult)
                    x2nf.append(t)
                # fp8 copy of the normalized activations for the experts
                for c in range(DCH // 2):
                    for jj in range(2):
                        nc.vector.tensor_scalar(x8[c][:, jj, :],
                                                x2nf[2 * c + jj],
                                                S_X, None, ALU.mult)
                # router scores in [E, TOK] layout: wr stationary
                scps = pp_r.tile([E, TOK], F32, tag="scp", name="scp",
                                 bufs=1)
                for dch in range(DCH):
                    nc.tensor.matmul(scps, sb_wrn[:, dch, :], x2nf[dch],
                                     start=(dch == 0), stop=(dch == DCH - 1))
                scs = p_sm.tile([E, TOK], F32, tag="scs", name="scs")
                nc.scalar.copy(scs, scps)
                tps = pp_r.tile([P, SL, E], F32, tag="tps", name="tps",
                                bufs=1)
                for c in range(SL):
                    nc.tensor.transpose(tps[:, c, :],
                                        scs[0:E, c * P:(c + 1) * P],
                                        sb_ident[0:E, 0:E])
                # gating math vectorized over all SL blocks at once
                sg = p_sm.tile([P, SL, E], F32, tag="sg", name="sg")
                nc.scalar.activation(sg, tps, AF.Sigmoid)
                tt = p_sm.tile([P, SL, E], F32, tag="tt", name="tt")
                nc.vector.tensor_tensor(
                    tt, sg,
                    sb_bias[:, :].unsqueeze(1).broadcast_to([P, SL, E]),
                    ALU.add)
                m1 = p_sm.tile([P, SL, 1], F32, tag="m1", name="m1")
                nc.vector.tensor_reduce(m1, tt, mybir.AxisListType.X,
                                        ALU.max)
                e1 = p_sm.tile([P, SL, E], F32, tag="e1", name="e1")
                nc.vector.tensor_tensor(e1, tt,
                                        m1.broadcast_to([P, SL, E]),
                                        ALU.is_ge)
                pen = p_sm.tile([P, SL, E], F32, tag="pen", name="pen")
                nc.vector.tensor_scalar(pen, e1, -1e9, None, ALU.mult)
                t2 = p_sm.tile([P, SL, E], F32, tag="t2", name="t2")
                nc.vector.tensor_tensor(t2, tt, pen, ALU.add)
                m2 = p_sm.tile([P, SL, 1], F32, tag="m2", name="m2")
                nc.vector.tensor_reduce(m2, t2, mybir.AxisListType.X,
                                        ALU.max)
                e2g = p_sm.tile([P, SL, E], F32, tag="e2g", name="e2g")
                nc.vector.tensor_tensor(e2g, t2,
                                        m2.broadcast_to([P, SL, E]),
                                        ALU.is_ge)
                sel = p_sm.tile([P, SL, E], F32, tag="sel", name="sel")
                nc.vector.tensor_tensor(sel, e1, e2g, ALU.add)
                gg = p_sm.tile([P, SL, E], F32, tag="gg", name="gg")
                nc.vector.tensor_tensor(gg, sg, sel, ALU.mult)
                dsum = p_sm.tile([P, SL, 1], F32, tag="dsum", name="dsum")
                nc.vector.tensor_reduce(dsum, gg, mybir.AxisListType.X,
                                        ALU.add)
                nc.vector.tensor_scalar(dsum, dsum, 1e-9, None, ALU.add)
                rcp = p_sm.tile([P, SL, 1], F32, tag="rcp", name="rcp")
                nc.vector.reciprocal(rcp, dsum)
                nc.vector.tensor_tensor(gg, gg,
                                        rcp.broadcast_to([P, SL, E]),
                                        ALU.mult)
                for c in range(SL):
                    gtp = pp_r.tile([E, P], F32, tag="gtp", name="gtp",
                                    bufs=2)
                    nc.tensor.transpose(gtp, gg[:, c, :], sb_ident)
                    nc.scalar.activation(gatesT[:, c * P:(c + 1) * P], gtp,
                                         AF.Copy, scale=C_GB)

            h2g = [p_moe.tile([P, 2, TOK], FP8, tag=f"h2g{e}",
                              name=f"h2g{e}") for e in range(E)]
            with ExitStack() as ectx:
                pp_hps = ectx.enter_context(
                    tc.tile_pool(name="phps", bufs=1, space="PSUM"))
                for e in range(E):
                    w13t = w13all[e]
                    if e + 3 < E:
                        em_w13(e + 3)
                    # hop the gate row to partition 0 first: the gpsimd
                    # broadcast reads partition 0 of its input AP
                    g1 = p_mw.tile([1, TOK], BF16, tag="g1", name="g1")
                    nc.sync.dma_start(g1, gatesT[e:e + 1, :])
                    gb = p_mw.tile([P, TOK], BF16, tag="gb", name="gb")
                    nc.gpsimd.partition_broadcast(gb, g1)
                    hpre = []
                    for m in range(4):
                        hps = pp_hps.tile([P, TOK], F32, tag=f"hps{m}",
                                          name=f"hps{m}")
                        for c in range(DCH // 2):
                            nc.tensor.matmul(
                                hps, w13t[:, c, :, m * P:(m + 1) * P],
                                x8[c],
                                start=(c == 0), stop=(c == DCH // 2 - 1),
                                perf_mode=mybir.MatmulPerfMode.DoubleRow)
                        hpre.append(hps)
                    for mc in range(2):
                        sa = p_sm.tile([P, TOK], BF16, tag="sa", name="sa")
                        nc.scalar.activation(sa, hpre[mc], AF.Silu,
                                             scale=C_SILU)
                        tg = p_sm.tile([P, TOK], BF16, tag="tg", name="tg")
                        nc.vector.tensor_tensor(tg, sa, hpre[mc + 2],
                                                ALU.mult)
                        nc.vector.tensor_tensor(h2g[e][:, mc, :], tg, gb,
                                                ALU.mult)

                with ExitStack() as w2ctx:
                    pp_yf = w2ctx.enter_context(
                        tc.tile_pool(name="pyf", bufs=2, space="PSUM"))
                    for dch in range(DCH):
                        yf = pp_yf.tile([P, TOK], F32, tag="yf", name="yf")
                        for e in range(E):
                            nc.tensor.matmul(
                                yf, w2all[e][:, :, dch * P:(dch + 1) * P],
                                h2g[e],
                                start=(e == 0), stop=(e == E - 1),
                                perf_mode=mybir.MatmulPerfMode.DoubleRow)
                        ot = p_sm.tile([P, TOK], F32, tag="ot", name="ot")
                        nc.vector.scalar_tensor_tensor(
                            ot, yf, C_OUT, x2T[dch], ALU.mult, ALU.add)
                        nc.sync.dma_start(outT[dch * P:(dch + 1) * P, :], ot)

    nc.compile()
    return nc


_NC_CACHE = {}


def _get_nc(S):
    if S not in _NC_CACHE:
        _NC_CACHE[S] = _build(S)
    return _NC_CACHE[S]


def _q8(v, s):
    return np.clip(v * s, -240.0, 240.0).astype(F8)


def host_prep(x, position_ids, norm1_w, wq, wdkv, wuk, wuv, wo,
              norm2_w, wr, router_bias, w1, w3, w2):
    x = np.asarray(x, np.float32)
    _, S, _ = x.shape
    NB = S // P
    SL = NB // 4

    pos = np.asarray(position_ids, np.int32)
    norm1_w = np.asarray(norm1_w, np.float32)
    norm2_w = np.asarray(norm2_w, np.float32)
    wq_n = (np.asarray(wq, np.float32) * norm1_w[:, None]).astype(BF)
    wdkv_n = (np.asarray(wdkv, np.float32) * norm1_w[:, None]).astype(BF)
    wuk_f = np.asarray(wuk, np.float32)
    # [R, 2, H*HD]: straight copy and per-head half-swapped copy so the
    # rope rotation needs no on-chip partition moves
    perm = np.concatenate([np.arange(h * HD + HALF, h * HD + HD).tolist() +
                           np.arange(h * HD, h * HD + HALF).tolist()
                           for h in range(H)]).astype(np.int64)
    wukx_b = np.ascontiguousarray(
        np.stack([wuk_f, wuk_f[:, perm]], axis=1)).astype(BF)
    wuv_b = np.asarray(wuv, np.float32).astype(BF)
    wo_b = np.asarray(wo, np.float32).astype(BF)
    wr_n = np.ascontiguousarray(np.asarray(wr, np.float32) * norm2_w[:, None])
    w13 = np.concatenate([np.asarray(w1, np.float32),
                          np.asarray(w3, np.float32)], axis=2)
    w13_n = w13 * norm2_w[None, :, None]
    # DoubleRow interleave: contract chunk c pairs d = c*256 + j*128 + ki
    w13_dr = np.ascontiguousarray(
        _q8(w13_n.reshape(E, 4, 2, P, 2 * MH).transpose(0, 1, 3, 2, 4),
            S_W13))
    w2_dr = np.ascontiguousarray(
        _q8(np.asarray(w2, np.float32).reshape(E, 2, P, D).transpose(
            0, 2, 1, 3), S_W2))
    bias_b = np.ascontiguousarray(np.broadcast_to(
        np.asarray(router_bias, np.float32)[None, :], (P, E)))
    ident = np.eye(P, dtype=np.float32)

    inv = 1.0 / (THETA ** (np.arange(HALF, dtype=np.float64) / HALF))

    in_maps = []
    slot_blocks_all = []
    for c in range(NCORES):
        b, r = divmod(c, 4)
        slot_blocks = [r + 4 * (SL - 1 - m) for m in range(SL)]
        slot_blocks_all.append(slot_blocks)
        own = np.concatenate(
            [np.arange(g * P, (g + 1) * P) for g in slot_blocks])

        ang = pos[b].astype(np.float64)[:, None] * inv[None, :]
        cosT = np.cos(ang).T.astype(np.float32)
        sinT = np.sin(ang).T.astype(np.float32)
        cos4k_h = np.tile(cosT, (4, 1)).astype(BF)
        sin4kn_h = np.concatenate([-sinT, sinT, -sinT, sinT], 0).astype(BF)
        cos4q_h = np.ascontiguousarray(cos4k_h[:, own])
        sin4qn_h = np.ascontiguousarray(sin4kn_h[:, own])

        xT_h = np.ascontiguousarray(x[b].T)
        xTq_h = np.ascontiguousarray(x[b].T[:, own])

        # additive mask: 0 where attention is allowed, -1e9 where masked
        maskt_h = np.full((NB, P, P), -1e9, np.float32)
        for j in range(NB):
            jm = j % 4
            if jm < r:
                maskt_h[j] = 0.0
            elif jm == r:
                maskt_h[j] = (np.triu(np.ones((P, P), np.float32))
                              - 1.0) * 1e9
        maskt_h = np.repeat(maskt_h[:, None, :, :], 2, axis=1).astype(BF)

        in_maps.append({
            "xT": xT_h, "xTq": xTq_h,
            "cos4k": cos4k_h, "sin4kn": sin4kn_h,
            "cos4q": cos4q_h, "sin4qn": sin4qn_h,
            "maskt": maskt_h,
            "wqn": wq_n, "wdkvn": wdkv_n, "wukx": wukx_b, "wuv": wuv_b,
            "wo": wo_b, "wrn": wr_n, "bias128": bias_b,
            "w13dr": w13_dr, "w2dr": w2_dr, "identf": ident,
        })
    return in_maps, slot_blocks_all


def run(inputs, trace=False):
    x = np.asarray(inputs["x"], np.float32)
    Bx, S, Dx = x.shape
    nc = _get_nc(S)
    in_maps, slot_blocks_all = host_prep(**inputs)
    res = run_bass_kernel_spmd(nc, in_maps, core_ids=list(range(NCORES)),
                               trace=trace)
    out = np.zeros((Bx, S, Dx), np.float32)
    for c in range(NCORES):
        b = c // 4
        oT = np.asarray(res.results[c]["outT"])
        for m, g in enumerate(slot_blocks_all[c]):
            out[b, g * P:(g + 1) * P, :] = oT[:, m * P:(m + 1) * P].T
    return out, res


def kernel(**inputs):
    out, _ = run(inputs)
    return out


# revision 76
# speedup vs baseline: 1.0333x; 1.0117x over previous
"""DeepSeekV3 block (MLA attention + top-2-of-8 MoE) on 8 trn2 NeuronCores.

Sharding: cores 0-3 -> batch 0, cores 4-7 -> batch 1. Within a batch group
of 4 cores, each core owns S/4 query tokens chosen as SL strided 128-row
blocks ordered by causal depth (blocks r+12, r+8, r+4, r for sub-rank r at
S=2048), which makes the flash-attention k-loop narrow uniformly across
cores: one SPMD program, all per-core differences live in input data.
k/v/latent projections are recomputed per core (replicated within the
batch group) to avoid collectives.

v2 changes vs baseline:
- q projection runs first (overlaps the xT window DMAs); window loop is
  double-buffered and fuses the per-window v up-projection.
- flash attention is software-pipelined: scores for block j+1 are issued
  ahead of the AV matmuls for block j so the in-order tensor queue never
  stalls on the softmax exp; the epilogue reciprocal runs directly on the
  PSUM row (no DMA round trip) and O2 is double-banked so the next head
  pair starts immediately.
- router matmul is reoriented (small wr stationary, tokens streamed) with
  the gating math vectorized over all four 128-token blocks at once.
- MoE expert matmuls run in fp8 (e4m3) with DoubleRow packing: weights are
  pre-scaled/interleaved on the host, activations are quantized on-chip,
  and all scale factors fold into existing activation/broadcast ops.
  Gate values are folded into the hidden states via a gpsimd partition
  broadcast instead of a tensor-engine broadcast matmul.

Layout convention: activations are kept transposed [feature, token] so
weight matrices are always the stationary matmul operand, and softmax
denominators come from a ones column appended to the value tiles.
"""

import sys

sys.path.insert(0, "/opt/trn_rl_repo")

from contextlib import ExitStack

import ml_dtypes
import numpy as np

import concourse.bass as bass
import concourse.tile as tile
from concourse import bacc
from concourse import mybir
from concourse.bass_utils import run_bass_kernel_spmd

F32 = mybir.dt.float32
BF16 = mybir.dt.bfloat16
FP8 = mybir.dt.float8e4
AF = mybir.ActivationFunctionType
ALU = mybir.AluOpType
BF = ml_dtypes.bfloat16
F8 = ml_dtypes.float8_e4m3

B, D = 2, 1024
H, HD = 16, 64
HALF = HD // 2
R = 256
E, TOPK, MH = 8, 2, 256
EPS = 1e-6
THETA = 10000.0
P = 128
NCORES = 8

# fp8 scale plan for the MoE: every scale is a power of two so the
# compensations fold exactly into activation scales.
S_X = 32.0        # x2n -> fp8
S_W13 = 2048.0    # w1/w3 -> fp8
S_W2 = 2048.0     # w2 -> fp8
S_H = 8.0         # hidden*gate -> fp8
C_SILU = 1.0 / (S_X * S_W13)          # descale inside the silu activation
C_GB = S_H / (S_X * S_W13)            # folded into the gate broadcast
C_OUT = 1.0 / (S_H * S_W2)            # final descale before the residual


def _build(S: int):
    NB = S // P               # seq blocks per batch (16 at S=2048)
    SL = NB // 4              # q-block slots per core
    TOK = SL * P              # own tokens per core
    WIN = min(512, S)
    NW = S // WIN
    NHP = H // 2              # 8 head pairs
    DCH = D // P              # 8
    RCH = R // P              # 2
    HD1 = HD + 1

    nc = bacc.Bacc(None, target_bir_lowering=False)

    xT = nc.dram_tensor("xT", [D, S], F32, kind="ExternalInput")
    xTq = nc.dram_tensor("xTq", [D, TOK], F32, kind="ExternalInput")
    cos4k = nc.dram_tensor("cos4k", [P, S], BF16, kind="ExternalInput")
    sin4kn = nc.dram_tensor("sin4kn", [P, S], BF16, kind="ExternalInput")
    cos4q = nc.dram_tensor("cos4q", [P, TOK], BF16, kind="ExternalInput")
    sin4qn = nc.dram_tensor("sin4qn", [P, TOK], BF16, kind="ExternalInput")
    maskt = nc.dram_tensor("maskt", [NB, 2, P, P], BF16, kind="ExternalInput")
    wqn = nc.dram_tensor("wqn", [D, H * HD], BF16, kind="ExternalInput")
    wdkvn = nc.dram_tensor("wdkvn", [D, R], BF16, kind="ExternalInput")
    wukx = nc.dram_tensor("wukx", [R, 2, H * HD], BF16, kind="ExternalInput")
    wuv = nc.dram_tensor("wuv", [R, H * HD], BF16, kind="ExternalInput")
    wo = nc.dram_tensor("wo", [H * HD, D], BF16, kind="ExternalInput")
    wrn = nc.dram_tensor("wrn", [D, E], F32, kind="ExternalInput")
    bias128 = nc.dram_tensor("bias128", [P, E], F32, kind="ExternalInput")
    w13dr = nc.dram_tensor("w13dr", [E, DCH // 2, P, 2, 2 * MH], FP8,
                           kind="ExternalInput")
    w2dr = nc.dram_tensor("w2dr", [E, P, 2, D], FP8, kind="ExternalInput")
    identf = nc.dram_tensor("identf", [P, P], F32, kind="ExternalInput")
    outT = nc.dram_tensor("outT", [D, TOK], F32, kind="ExternalOutput")

    with tile.TileContext(nc) as tc, ExitStack() as ctx:
        p_const = ctx.enter_context(tc.tile_pool(name="const", bufs=1))
        p_x2 = ctx.enter_context(tc.tile_pool(name="x2", bufs=1))

        ones_bf = p_const.tile([P, 1], BF16, tag="ones_bf", name="ones_bf")
        nc.vector.memset(ones_bf, 1.0)
        eps1 = p_const.tile([1, 1], F32, tag="eps1", name="eps1")
        nc.vector.memset(eps1, EPS)
        sb_ident = p_const.tile([P, P], F32, tag="ident", name="ident")
        nc.sync.dma_start(sb_ident, identf[:, :])
        ident_bf = p_const.tile([P, P], BF16, tag="identb", name="identb")
        nc.scalar.copy(ident_bf, sb_ident)

        # own-token x loads first: the q projection consumes them and runs
        # while the full-sequence windows stream in behind.
        sb_xq = []
        for dch in range(DCH):
            t = p_x2.tile([P, TOK], F32, tag=f"xq{dch}", name=f"xq{dch}")
            nc.sync.dma_start(t, xTq[dch * P:(dch + 1) * P, :])
            sb_xq.append(t)

        # all big loads share the sync queue in explicit first-use order;
        # tiles are declared here, the DMAs are interleaved below
        p_wk = ctx.enter_context(tc.tile_pool(name="wk", bufs=1))
        sb_wdkv = p_wk.tile([P, DCH, R], BF16, tag="wdkv", name="wdkv")
        sb_wuv = p_wk.tile([P, RCH, H * HD], BF16, tag="wuv", name="wuv")
        sb_wuk = p_wk.tile([P, RCH, 2, H * HD], BF16, tag="wuk", name="wuk")
        sb_cos4k = p_wk.tile([P, S], BF16, tag="cos4k", name="cos4k")
        sb_sin4kn = p_wk.tile([P, S], BF16, tag="sin4kn", name="sin4kn")

        def rmsnorm_cols(pool, ppool, src_tiles, ncols, nametag):
            """src_tiles: DCH sbuf [P, ncols] f32 -> DCH bf16 tiles,
            rms-normalized across the full d axis. Also returns the psum
            broadcast of 1/rms for fp32 consumers."""
            sq = []
            for dch in range(DCH):
                t = pool.tile([P, ncols], BF16, tag=f"sq{dch % 2}",
                              name=f"sq{dch % 2}")
                nc.scalar.activation(t, src_tiles[dch], AF.Square)
                sq.append(t)
            ss = ppool.tile([P, ncols], F32, tag="ss", name="ss")
            for dch in range(DCH):
                nc.tensor.matmul(ss[0:1, :], ones_bf, sq[dch],
                                 start=(dch == 0), stop=(dch == DCH - 1))
            sd = pool.tile([1, ncols], F32, tag="sd", name="sd")
            nc.scalar.activation(sd, ss[0:1, :], AF.Sqrt,
                                 bias=eps1, scale=1.0 / D)
            sdw = pool.tile([P, ncols // P], F32, tag="sdw", name="sdw")
            nc.sync.dma_start(sdw, sd)
            rcw = pool.tile([P, ncols // P], F32, tag="rcw", name="rcw")
            nc.vector.reciprocal(rcw, sdw)
            rsv = pool.tile([1, ncols], F32, tag="rsv", name="rsv")
            nc.sync.dma_start(rsv, rcw)
            rsb = pool.tile([P, ncols], F32, tag="rsb_s", name="rsb_s")
            nc.gpsimd.partition_broadcast(rsb, rsv)
            out = []
            for dch in range(DCH):
                t = pool.tile([P, ncols], BF16, tag=f"h_{nametag}{dch}",
                              name=f"h_{nametag}{dch}")
                nc.vector.tensor_tensor(t, src_tiles[dch], rsb, ALU.mult)
                out.append(t)
            return out, rsb

        def rope6(pool, pre_ps, cos_t, sin_t, out_tile, nametag):
            """rope on psum [P, cols] (2 heads stacked) -> bf16 out_tile.
            Engines are partition-lane-locked, so the half-swap goes
            through SBUF->SBUF DMA."""
            kbf = pool.tile(list(out_tile.shape), BF16, tag=f"rkb_{nametag}",
                            name=f"rkb_{nametag}")
            nc.scalar.copy(kbf, pre_ps)
            ksw = pool.tile(list(out_tile.shape), BF16, tag=f"rsw_{nametag}",
                            name=f"rsw_{nametag}")
            # gpsimd queue: keeps the partition swap off the sync queue,
            # which carries the latency-critical input loads
            for g in range(4):
                a = g * HALF
                pa = (g + 1) * HALF if g % 2 == 0 else (g - 1) * HALF
                nc.gpsimd.dma_start(ksw[a:a + HALF, :], kbf[pa:pa + HALF, :])
            tmp = pool.tile(list(out_tile.shape), BF16, tag=f"rtm_{nametag}",
                            name=f"rtm_{nametag}")
            nc.vector.tensor_tensor(tmp, ksw, sin_t, ALU.mult)
            nc.vector.tensor_tensor(out_tile, kbf, cos_t, ALU.mult)
            nc.vector.tensor_tensor(out_tile, out_tile, tmp, ALU.add)

        attnT = [p_x2.tile([P, TOK], BF16, tag=f"attnT{i}", name=f"attnT{i}")
                 for i in range(NHP)]
        qTa = [p_x2.tile([P, TOK], BF16, tag=f"qTa{i}", name=f"qTa{i}")
               for i in range(NHP)]

        with ExitStack() as kvctx:
            p_kv = kvctx.enter_context(tc.tile_pool(name="kv", bufs=1))
            vext = [p_kv.tile([P, H * HD1], BF16, tag=f"vext{i}",
                              name=f"vext{i}") for i in range(NB)]
            cT = [p_kv.tile([P, S], BF16, tag=f"cT{i}", name=f"cT{i}")
                  for i in range(RCH)]

            # ---- phase A: qT + rope (own tokens; overlaps window DMAs) ----
            with ExitStack() as s5:
                p_q = s5.enter_context(tc.tile_pool(name="q", bufs=2))
                p_wq = s5.enter_context(tc.tile_pool(name="wqp", bufs=1))
                pp_5 = s5.enter_context(
                    tc.tile_pool(name="p5", bufs=2, space="PSUM"))
                sb_wq = p_wq.tile([P, DCH, H * HD], BF16, tag="wq", name="wq")
                nc.sync.dma_start(
                    sb_wq, wqn[:, :].rearrange("(c p) n -> p c n", p=P))
                sb_cos4q = p_wq.tile([P, TOK], BF16, tag="cos4q",
                                     name="cos4q")
                nc.sync.dma_start(sb_cos4q, cos4q[:, :])
                sb_sin4qn = p_wq.tile([P, TOK], BF16, tag="sin4qn",
                                      name="sin4qn")
                nc.sync.dma_start(sb_sin4qn, sin4qn[:, :])
                h1q, _ = rmsnorm_cols(p_q, pp_5, sb_xq, TOK, "nq")
                for hp in range(NHP):
                    hc = hp * 2 * HD
                    qps = pp_5.tile([P, TOK], F32, tag="mm", name="mm")
                    for dch in range(DCH):
                        nc.tensor.matmul(
                            qps, sb_wq[:, dch, hc:hc + P], h1q[dch],
                            start=(dch == 0), stop=(dch == DCH - 1))
                    rope6(p_q, qps, sb_cos4q, sb_sin4qn, qTa[hp], "q")

            p_kt = kvctx.enter_context(tc.tile_pool(name="kt", bufs=2))

            def em_kt_win(kt, khp, w, ppool, ptag, pbufs):
                hc = khp * 2 * HD
                c0 = w * WIN
                if pbufs is None:
                    kps = ppool.tile([P, 2, 512], F32, tag=ptag, name=ptag)
                else:
                    kps = ppool.tile([P, 2, 512], F32, tag=ptag, name=ptag,
                                     bufs=pbufs)
                # sw=0: k, sw=1: half-swapped k (weights permuted
                # host-side) -> rope without any partition moves
                for sw in range(2):
                    for rch in range(RCH):
                        nc.tensor.matmul(
                            kps[:, sw, 0:WIN],
                            sb_wuk[:, rch, sw, hc:hc + P],
                            cT[rch][:, c0:c0 + WIN],
                            start=(rch == 0), stop=(rch == RCH - 1))
                kbf2 = p_kt.tile([P, 2, WIN], BF16, tag="kbf2", name="kbf2")
                nc.vector.tensor_copy(kbf2, kps[:, :, 0:WIN])
                ktmp = p_kt.tile([P, WIN], BF16, tag="ktmp", name="ktmp")
                nc.vector.tensor_tensor(ktmp, kbf2[:, 1, :],
                                        sb_sin4kn[:, c0:c0 + WIN], ALU.mult)
                nc.vector.tensor_tensor(kt[:, c0:c0 + WIN], kbf2[:, 0, :],
                                        sb_cos4k[:, c0:c0 + WIN], ALU.mult)
                nc.vector.tensor_tensor(kt[:, c0:c0 + WIN],
                                        kt[:, c0:c0 + WIN], ktmp, ALU.add)

            kt0 = p_kt.tile([P, S], BF16, tag="kTa", name="kTa")

            # ---- phase B: per window rmsnorm -> latent cT -> v up;
            # head pair 0's kT is built as each window's cT lands ----
            with ExitStack() as s12:
                p_xw = s12.enter_context(tc.tile_pool(name="xw", bufs=3))
                p_n1 = s12.enter_context(tc.tile_pool(name="n1", bufs=2))
                pp_12 = s12.enter_context(
                    tc.tile_pool(name="p12", bufs=2, space="PSUM"))

                def em_xw(w):
                    c0 = w * WIN
                    xw = []
                    for dch in range(DCH):
                        t = p_xw.tile([P, WIN], F32, tag=f"xw{dch}",
                                      name=f"xw{dch}")
                        nc.sync.dma_start(
                            t, xT[dch * P:(dch + 1) * P, c0:c0 + WIN])
                        xw.append(t)
                    return xw

                # windows 0/1 load right behind the phase-A inputs; the
                # flash-phase weights are interleaved between the remaining
                # windows so everything lands just before first use
                xw_pre = [em_xw(0), em_xw(1)]
                nc.sync.dma_start(
                    sb_wdkv, wdkvn[:, :].rearrange("(c p) r -> p c r", p=P))
                nc.sync.dma_start(
                    sb_wuv, wuv[:, :].rearrange("(c p) n -> p c n", p=P))
                nc.sync.dma_start(
                    sb_wuk,
                    wukx[:, :, :].rearrange("(c p) s n -> p c s n", p=P))
                for w in range(NW):
                    c0 = w * WIN
                    if w < 2:
                        xw = xw_pre[w]
                    else:
                        xw = em_xw(w)
                    if w == 1:
                        nc.sync.dma_start(sb_cos4k, cos4k[:, :])
                        nc.sync.dma_start(sb_sin4kn, sin4kn[:, :])
                    h1w, _ = rmsnorm_cols(p_n1, pp_12, xw, WIN, "n1")
                    for rch in range(RCH):
                        cps = pp_12.tile([P, WIN], F32, tag="mm", name="mm")
                        for dch in range(DCH):
                            nc.tensor.matmul(
                                cps, sb_wdkv[:, dch, rch * P:(rch + 1) * P],
                                h1w[dch],
                                start=(dch == 0), stop=(dch == DCH - 1))
                        nc.scalar.copy(cT[rch][:, c0:c0 + WIN], cps)
                    for tb in range(w * (WIN // P), (w + 1) * (WIN // P)):
                        for nh in range(2):
                            vps = pp_12.tile([P, 512], F32, tag="mm",
                                             name="mm")
                            for rch in range(RCH):
                                nc.tensor.matmul(
                                    vps, cT[rch][:, tb * P:(tb + 1) * P],
                                    sb_wuv[:, rch, nh * 512:(nh + 1) * 512],
                                    start=(rch == 0), stop=(rch == RCH - 1))
                            dst = vext[tb][:, :].rearrange(
                                "p (h s) -> p h s", s=HD1)
                            nc.scalar.copy(
                                dst[:, nh * 8:(nh + 1) * 8, 0:HD],
                                vps[:, :].rearrange("p (h s) -> p h s", s=HD))
                        oc = vext[tb][:, :].rearrange(
                            "p (h s) -> p h s", s=HD1)[:, :, HD:HD1]
                        nc.vector.memset(oc, 1.0)

            # ---- phase C: per head pair, kT+rope then pipelined flash ----
            x2T = [p_x2.tile([P, TOK], F32, tag=f"x2T{i}", name=f"x2T{i}")
                   for i in range(DCH)]
            # group the causal blocks: equal-N tail blocks share one PSUM
            # tile and one exp activation to amortize per-op overhead
            jgroups = ([[j] for j in range(8)]
                       + [[8, 9], [10, 11], [12, 13, 14, 15]])

            with ExitStack() as s6:
                p_fl = s6.enter_context(tc.tile_pool(name="fl", bufs=2))
                p_wo2 = s6.enter_context(tc.tile_pool(name="wop", bufs=1))
                pp_fl = s6.enter_context(
                    tc.tile_pool(name="pfl", bufs=2, space="PSUM"))
                sb_mask = p_wo2.tile([P, NB, 2, P], BF16, tag="mask",
                                     name="mask")
                nc.sync.dma_start(
                    sb_mask, maskt[:, :, :, :].rearrange("j g k q -> k j g q"))
                sb_wo = p_wo2.tile([P, DCH, D], BF16, tag="wo", name="wo")
                nc.sync.dma_start(
                    sb_wo, wo[:, :].rearrange("(c p) n -> p c n", p=P))

                def em_scores(kt, hp, grp):
                    N = (SL - grp[0] // 4) * P
                    s2 = pp_fl.tile([P, 2, 512], F32, tag="s2", name="s2")
                    for gi, j in enumerate(grp):
                        o = gi * N
                        jc = slice(j * P, (j + 1) * P)
                        nc.tensor.matmul(s2[:, 0, o:o + N], kt[0:HD, jc],
                                         qTa[hp][0:HD, 0:N],
                                         start=True, stop=False,
                                         skip_group_check=True)
                        nc.tensor.matmul(s2[:, 1, o:o + N], kt[HD:P, jc],
                                         qTa[hp][HD:P, 0:N],
                                         start=True, stop=False,
                                         skip_group_check=True)
                        # additive causal mask folded into the PSUM via
                        # identity-stationary matmul (-1e9 when masked)
                        nc.tensor.matmul(s2[:, :, o + N - P:o + N],
                                         ident_bf, sb_mask[:, j, :, :],
                                         start=False, stop=True,
                                         skip_group_check=True)
                    return s2

                kt_cur = kt0
                for w in range(NW):
                    em_kt_win(kt_cur, 0, w, pp_fl, "s2", None)
                for hp in range(NHP):
                    # next head pair's kT builds *inside* this head pair's
                    # flash loop: the rope vector work hides under the
                    # scores/AV matmuls instead of bunching at the boundary
                    kt_next = (p_kt.tile([P, S], BF16, tag="kTa", name="kTa")
                               if hp + 1 < NHP else None)

                    O2 = pp_fl.tile([P, 2, 512], F32, tag="O2", name="O2")
                    s2p = em_scores(kt_cur, hp, jgroups[0])
                    for gidx, grp in enumerate(jgroups):
                        N = (SL - grp[0] // 4) * P
                        G = len(grp)
                        s2n = (em_scores(kt_cur, hp, jgroups[gidx + 1])
                               if gidx < len(jgroups) - 1 else None)
                        if kt_next is not None and gidx in (2, 4, 6, 8):
                            em_kt_win(kt_next, hp + 1, (gidx - 2) // 2,
                                      pp_fl, "s2", None)
                        e2 = p_fl.tile([P, 2, 512], BF16, tag="e2",
                                       name="e2", bufs=3)
                        nc.scalar.activation(e2[:, :, 0:G * N],
                                             s2p[:, :, 0:G * N],
                                             AF.Exp, scale=0.125)
                        for gi, j in enumerate(grp):
                            o = gi * N
                            ve = vext[j][:, :].rearrange("p (h s) -> p h s",
                                                         s=HD1)
                            nc.tensor.matmul(
                                O2[0:HD1, 0, 0:N], ve[:, hp * 2, :],
                                e2[:, 0, o:o + N], start=(j == 0),
                                stop=(j == NB - 1), skip_group_check=True)
                            nc.tensor.matmul(
                                O2[0:HD1, 1, 0:N], ve[:, hp * 2 + 1, :],
                                e2[:, 1, o:o + N], start=(j == 0),
                                stop=(j == NB - 1), skip_group_check=True)
                        s2p = s2n
                    # normalize: attnT = O[0:64] * (1/l); l sits in row 64
                    sums = p_fl.tile([1, 2, TOK], F32, tag="sums",
                                     name="sums")
                    nc.scalar.copy(sums, O2[HD:HD1, :, 0:TOK])
                    sw = p_fl.tile([P, 2 * TOK // P], F32, tag="sw",
                                   name="sw")
                    nc.sync.dma_start(sw, sums)
                    rw = p_fl.tile([P, 2 * TOK // P], F32, tag="rw",
                                   name="rw")
                    nc.vector.reciprocal(rw, sw)
                    linv = p_fl.tile([1, 2, TOK], F32, tag="linv",
                                     name="linv")
                    nc.sync.dma_start(linv, rw)
                    lps = p_fl.tile([HD, 2, TOK], F32, tag="lps", name="lps")
                    nc.gpsimd.partition_broadcast(lps, linv)
                    onum = p_fl.tile([HD, 2, TOK], BF16, tag="onum",
                                     name="onum")
                    nc.scalar.copy(onum, O2[0:HD, :, 0:TOK])
                    nc.vector.tensor_tensor(attnT[hp][0:HD, :], onum[:, 0, :],
                                            lps[:, 0, :], ALU.mult)
                    a2 = p_fl.tile([HD, TOK], BF16, tag="a2", name="a2")
                    nc.vector.tensor_tensor(a2, onum[:, 1, :],
                                            lps[:, 1, :], ALU.mult)
                    # head 2 belongs on partitions 64:128 -> move via DMA
                    nc.sync.dma_start(attnT[hp][HD:P, :], a2)
                    kt_cur = kt_next

                # wo projection + residual -> x2T
                for dch in range(DCH):
                    yps = pp_fl.tile([P, 2, 512], F32, tag="s2", name="s2")
                    for hch in range(DCH):
                        nc.tensor.matmul(
                            yps[:, 0, :],
                            sb_wo[:, hch, dch * P:(dch + 1) * P],
                            attnT[hch],
                            start=(hch == 0), stop=(hch == DCH - 1))
                    nc.vector.tensor_tensor(x2T[dch], yps[:, 0, :],
                                            sb_xq[dch], ALU.add)

        # ================= router + MoE =================
        with ExitStack() as mctx:
            p_moe = mctx.enter_context(tc.tile_pool(name="moe", bufs=1))
            p_sm = mctx.enter_context(tc.tile_pool(name="sm", bufs=2))

            sb_wrn = p_moe.tile([P, DCH, E], F32, tag="wrn", name="wrn")
            nc.sync.dma_start(sb_wrn,
                              wrn[:, :].rearrange("(c p) e -> p c e", p=P))
            sb_bias = p_moe.tile([P, E], F32, tag="bias", name="bias")
            nc.sync.dma_start(sb_bias, bias128[:, :])
            gatesT = p_moe.tile([E, TOK], BF16, tag="gatesT", name="gatesT")
            x8 = [p_moe.tile([P, 2, TOK], FP8, tag=f"x8_{c}", name=f"x8_{c}")
                  for c in range(DCH // 2)]
            # prefetch the expert weights: w2 (all 8) and the first w13s
            # stream in during the router computation
            p_mw = mctx.enter_context(tc.tile_pool(name="mw", bufs=3))
            p_w2 = mctx.enter_context(tc.tile_pool(name="w2p", bufs=1))
            w2all = []
            for e in range(E):
                t = p_w2.tile([P, 2, D], FP8, tag=f"w2_{e}", name=f"w2_{e}")
                nc.sync.dma_start(t, w2dr[e, :, :, :])
                w2all.append(t)
            w13all = []

            def em_w13(e):
                t = p_mw.tile([P, DCH // 2, 2, 2 * MH], FP8,
                              tag="w13t", name="w13t")
                nc.sync.dma_start(t, w13dr[e, :, :, :, :].rearrange(
                    "c p j n -> p c j n"))
                w13all.append(t)

            # only as many prefetches as there are buffers: a deeper queue
            # would block the sync engine on buffer recycling
            for e in range(3):
                em_w13(e)
            with ExitStack() as rctx:
                pp_r = rctx.enter_context(
                    tc.tile_pool(name="pr", bufs=2, space="PSUM"))
                # rmsnorm of x2 (fp32 path kept exact for the router)
                sq2 = []
                for dch in range(DCH):
                    t = p_sm.tile([P, TOK], BF16, tag=f"sq{dch % 2}",
                                  name=f"sq{dch % 2}")
                    nc.scalar.activation(t, x2T[dch], AF.Square)
                    sq2.append(t)
                ss2 = pp_r.tile([P, TOK], F32, tag="ss", name="ss", bufs=1)
                for dch in range(DCH):
                    nc.tensor.matmul(ss2[0:1, :], ones_bf, sq2[dch],
                                     start=(dch == 0), stop=(dch == DCH - 1))
                sd2 = p_sm.tile([1, TOK], F32, tag="sd", name="sd")
                nc.scalar.activation(sd2, ss2[0:1, :], AF.Sqrt,
                                     bias=eps1, scale=1.0 / D)
                sdw2 = p_sm.tile([P, TOK // P], F32, tag="sdw", name="sdw")
                nc.sync.dma_start(sdw2, sd2)
                rcw2 = p_sm.tile([P, TOK // P], F32, tag="rcw", name="rcw")
                nc.vector.reciprocal(rcw2, sdw2)
                rsv2 = p_sm.tile([1, TOK], F32, tag="rsv", name="rsv")
                nc.sync.dma_start(rsv2, rcw2)
                rsb2 = p_sm.tile([P, TOK], F32, tag="rsb2", name="rsb2",
                                 bufs=1)
                nc.gpsimd.partition_broadcast(rsb2, rsv2)
                x2nf = []
                for dch in range(DCH):
                    t = p_moe.tile([P, TOK], F32, tag=f"x2nf{dch}",
                                   name=f"x2nf{dch}")
                    nc.vector.tensor_tensor(t, x2T[dch], rsb2, ALU.mult)
                    x2nf.append(t)
                # fp8 copy of the normalized activations for the experts
                for c in range(DCH // 2):
                    for jj in range(2):
                        nc.vector.tensor_scalar(x8[c][:, jj, :],
                                                x2nf[2 * c + jj],
                                                S_X, None, ALU.mult)
                # router scores in [E, TOK] layout: wr stationary
                scps = pp_r.tile([E, TOK], F32, tag="scp", name="scp",
                                 bufs=1)
                for dch in range(DCH):
                    nc.tensor.matmul(scps, sb_wrn[:, dch, :], x2nf[dch],
                                     start=(dch == 0), stop=(dch == DCH - 1))
                scs = p_sm.tile([E, TOK], F32, tag="scs", name="scs")
                nc.scalar.copy(scs, scps)
                tps = pp_r.tile([P, SL, E], F32, tag="tps", name="tps",
                                bufs=1)
                for c in range(SL):
                    nc.tensor.transpose(tps[:, c, :],
                                        scs[0:E, c * P:(c + 1) * P],
                                        sb_ident[0:E, 0:E])
                # gating math vectorized over all SL blocks at once
                sg = p_sm.tile([P, SL, E], F32, tag="sg", name="sg")
                nc.scalar.activation(sg, tps, AF.Sigmoid)
                tt = p_sm.tile([P, SL, E], F32, tag="tt", name="tt")
                nc.vector.tensor_tensor(
                    tt, sg,
                    sb_bias[:, :].unsqueeze(1).broadcast_to([P, SL, E]),
                    ALU.add)
                m1 = p_sm.tile([P, SL, 1], F32, tag="m1", name="m1")
                nc.vector.tensor_reduce(m1, tt, mybir.AxisListType.X,
                                        ALU.max)
                e1 = p_sm.tile([P, SL, E], F32, tag="e1", name="e1")
                nc.vector.tensor_tensor(e1, tt,
                                        m1.broadcast_to([P, SL, E]),
                                        ALU.is_ge)
                pen = p_sm.tile([P, SL, E], F32, tag="pen", name="pen")
                nc.vector.tensor_scalar(pen, e1, -1e9, None, ALU.mult)
                t2 = p_sm.tile([P, SL, E], F32, tag="t2", name="t2")
                nc.vector.tensor_tensor(t2, tt, pen, ALU.add)
                m2 = p_sm.tile([P, SL, 1], F32, tag="m2", name="m2")
                nc.vector.tensor_reduce(m2, t2, mybir.AxisListType.X,
                                        ALU.max)
                e2g = p_sm.tile([P, SL, E], F32, tag="e2g", name="e2g")
                nc.vector.tensor_tensor(e2g, t2,
                                        m2.broadcast_to([P, SL, E]),
                                        ALU.is_ge)
                sel = p_sm.tile([P, SL, E], F32, tag="sel", name="sel")
                nc.vector.tensor_tensor(sel, e1, e2g, ALU.add)
                gg = p_sm.tile([P, SL, E], F32, tag="gg", name="gg")
                nc.vector.tensor_tensor(gg, sg, sel, ALU.mult)
                dsum = p_sm.tile([P, SL, 1], F32, tag="dsum", name="dsum")
                nc.vector.tensor_reduce(dsum, gg, mybir.AxisListType.X,
                                        ALU.add)
                nc.vector.tensor_scalar(dsum, dsum, 1e-9, None, ALU.add)
                rcp = p_sm.tile([P, SL, 1], F32, tag="rcp", name="rcp")
                nc.vector.reciprocal(rcp, dsum)
                nc.vector.tensor_tensor(gg, gg,
                                        rcp.broadcast_to([P, SL, E]),
                                        ALU.mult)
                for c in range(SL):
                    gtp = pp_r.tile([E, P], F32, tag="gtp", name="gtp",
                                    bufs=2)
                    nc.tensor.transpose(gtp, gg[:, c, :], sb_ident)
                    nc.scalar.activation(gatesT[:, c * P:(c + 1) * P], gtp,
                                         AF.Copy, scale=C_GB)

            h2g = [p_moe.tile([P, 2, TOK], FP8, tag=f"h2g{e}",
                              name=f"h2g{e}") for e in range(E)]
            with ExitStack() as ectx:
                pp_hps = ectx.enter_context(
                    tc.tile_pool(name="phps", bufs=1, space="PSUM"))
                for e in range(E):
                    w13t = w13all[e]
                    if e + 3 < E:
                        em_w13(e + 3)
                    # hop the gate row to partition 0 first: the gpsimd
                    # broadcast reads partition 0 of its input AP
                    g1 = p_mw.tile([1, TOK], BF16, tag="g1", name="g1")
                    nc.sync.dma_start(g1, gatesT[e:e + 1, :])
                    gb = p_mw.tile([P, TOK], BF16, tag="gb", name="gb")
                    nc.gpsimd.partition_broadcast(gb, g1)
                    hpre = []
                    for m in range(4):
                        hps = pp_hps.tile([P, TOK], F32, tag=f"hps{m}",
                                          name=f"hps{m}")
                        for c in range(DCH // 2):
                            nc.tensor.matmul(
                                hps, w13t[:, c, :, m * P:(m + 1) * P],
                                x8[c],
                                start=(c == 0), stop=(c == DCH // 2 - 1),
                                perf_mode=mybir.MatmulPerfMode.DoubleRow)
                        hpre.append(hps)
                    for mc in range(2):
                        sa = p_sm.tile([P, TOK], BF16, tag="sa", name="sa")
                        nc.scalar.activation(sa, hpre[mc], AF.Silu,
                                             scale=C_SILU)
                        tg = p_sm.tile([P, TOK], BF16, tag="tg", name="tg")
                        nc.vector.tensor_tensor(tg, sa, hpre[mc + 2],
                                                ALU.mult)
                        nc.vector.tensor_tensor(h2g[e][:, mc, :], tg, gb,
                                                ALU.mult)

                with ExitStack() as w2ctx:
                    pp_yf = w2ctx.enter_context(
                        tc.tile_pool(name="pyf", bufs=2, space="PSUM"))
                    for dch in range(DCH):
                        yf = pp_yf.tile([P, TOK], F32, tag="yf", name="yf")
                        for e in range(E):
                            nc.tensor.matmul(
                                yf, w2all[e][:, :, dch * P:(dch + 1) * P],
                                h2g[e],
                                start=(e == 0), stop=(e == E - 1),
                                perf_mode=mybir.MatmulPerfMode.DoubleRow)
                        ot = p_sm.tile([P, TOK], F32, tag="ot", name="ot")
                        nc.vector.scalar_tensor_tensor(
                            ot, yf, C_OUT, x2T[dch], ALU.mult, ALU.add)
                        nc.sync.dma_start(outT[dch * P:(dch + 1) * P, :], ot)

    nc.compile()
    return nc


_NC_CACHE = {}


def _get_nc(S):
    if S not in _NC_CACHE:
        _NC_CACHE[S] = _build(S)
    return _NC_CACHE[S]


def _q8(v, s):
    return np.clip(v * s, -240.0, 240.0).astype(F8)


def host_prep(x, position_ids, norm1_w, wq, wdkv, wuk, wuv, wo,
              norm2_w, wr, router_bias, w1, w3, w2):
    x = np.asarray(x, np.float32)
    _, S, _ = x.shape
    NB = S // P
    SL = NB // 4

    pos = np.asarray(position_ids, np.int32)
    norm1_w = np.asarray(norm1_w, np.float32)
    norm2_w = np.asarray(norm2_w, np.float32)
    wq_n = (np.asarray(wq, np.float32) * norm1_w[:, None]).astype(BF)
    wdkv_n = (np.asarray(wdkv, np.float32) * norm1_w[:, None]).astype(BF)
    wuk_f = np.asarray(wuk, np.float32)
    # [R, 2, H*HD]: straight copy and per-head half-swapped copy so the
    # rope rotation needs no on-chip partition moves
    perm = np.concatenate([np.arange(h * HD + HALF, h * HD + HD).tolist() +
                           np.arange(h * HD, h * HD + HALF).tolist()
                           for h in range(H)]).astype(np.int64)
    wukx_b = np.ascontiguousarray(
        np.stack([wuk_f, wuk_f[:, perm]], axis=1)).astype(BF)
    wuv_b = np.asarray(wuv, np.float32).astype(BF)
    wo_b = np.asarray(wo, np.float32).astype(BF)
    wr_n = np.ascontiguousarray(np.asarray(wr, np.float32) * norm2_w[:, None])
    w13 = np.concatenate([np.asarray(w1, np.float32),
                          np.asarray(w3, np.float32)], axis=2)
    w13_n = w13 * norm2_w[None, :, None]
    # DoubleRow interleave: contract chunk c pairs d = c*256 + j*128 + ki
    w13_dr = np.ascontiguousarray(
        _q8(w13_n.reshape(E, 4, 2, P, 2 * MH).transpose(0, 1, 3, 2, 4),
            S_W13))
    w2_dr = np.ascontiguousarray(
        _q8(np.asarray(w2, np.float32).reshape(E, 2, P, D).transpose(
            0, 2, 1, 3), S_W2))
    bias_b = np.ascontiguousarray(np.broadcast_to(
        np.asarray(router_bias, np.float32)[None, :], (P, E)))
    ident = np.eye(P, dtype=np.float32)

    inv = 1.0 / (THETA ** (np.arange(HALF, dtype=np.float64) / HALF))

    in_maps = []
    slot_blocks_all = []
    for c in range(NCORES):
        b, r = divmod(c, 4)
        slot_blocks = [r + 4 * (SL - 1 - m) for m in range(SL)]
        slot_blocks_all.append(slot_blocks)
        own = np.concatenate(
            [np.arange(g * P, (g + 1) * P) for g in slot_blocks])

        ang = pos[b].astype(np.float64)[:, None] * inv[None, :]
        cosT = np.cos(ang).T.astype(np.float32)
        sinT = np.sin(ang).T.astype(np.float32)
        cos4k_h = np.tile(cosT, (4, 1)).astype(BF)
        sin4kn_h = np.concatenate([-sinT, sinT, -sinT, sinT], 0).astype(BF)
        cos4q_h = np.ascontiguousarray(cos4k_h[:, own])
        sin4qn_h = np.ascontiguousarray(sin4kn_h[:, own])

        xT_h = np.ascontiguousarray(x[b].T)
        xTq_h = np.ascontiguousarray(x[b].T[:, own])

        # additive mask: 0 where attention is allowed, -1e9 where masked
        maskt_h = np.full((NB, P, P), -1e9, np.float32)
        for j in range(NB):
            jm = j % 4
            if jm < r:
                maskt_h[j] = 0.0
            elif jm == r:
                maskt_h[j] = (np.triu(np.ones((P, P), np.float32))
                              - 1.0) * 1e9
        maskt_h = np.repeat(maskt_h[:, None, :, :], 2, axis=1).astype(BF)

        in_maps.append({
            "xT": xT_h, "xTq": xTq_h,
            "cos4k": cos4k_h, "sin4kn": sin4kn_h,
            "cos4q": cos4q_h, "sin4qn": sin4qn_h,
            "maskt": maskt_h,
            "wqn": wq_n, "wdkvn": wdkv_n, "wukx": wukx_b, "wuv": wuv_b,
            "wo": wo_b, "wrn": wr_n, "bias128": bias_b,
            "w13dr": w13_dr, "w2dr": w2_dr, "identf": ident,
        })
    return in_maps, slot_blocks_all


def run(inputs, trace=False):
    x = np.asarray(inputs["x"], np.float32)
    Bx, S, Dx = x.shape
    nc = _get_nc(S)
    in_maps, slot_blocks_all = host_prep(**inputs)
    res = run_bass_kernel_spmd(nc, in_maps, core_ids=list(range(NCORES)),
                               trace=trace)
    out = np.zeros((Bx, S, Dx), np.float32)
    for c in range(NCORES):
        b = c // 4
        oT = np.asarray(res.results[c]["outT"])
        for m, g in enumerate(slot_blocks_all[c]):
            out[b, g * P:(g + 1) * P, :] = oT[:, m * P:(m + 1) * P].T
    return out, res


def kernel(**inputs):
    out, _ = run(inputs)
    return out
